# revision 17
# baseline (speedup 1.0000x reference)
"""Trainium2 Bass kernel for a ViT-style transformer block (nn_Block).

Reference computation (per batch sequence):
    h   = LN(x) * g1 + b1
    qkv = h @ w_qkv ; attention (12 heads, dh=64, softmax over keys)
    x   = x + (attn_out @ w_proj + b_proj)
    h2  = LN(x) * g2 + b2
    out = x + gelu(h2 @ w1 + b1) @ w2 + b2

Sharding: 8 cores; core c handles half of sequence b=c//2 (1024 query
tokens).  The input sequence is host-rolled so each core's own tokens are
always rows [0:1024] (keeps the SPMD program identical across cores).
K/V are computed on-device for the full 2048-token sequence (redundantly
per core pair) so no collectives are needed.

Layout strategy (all matmul operands bf16, fp32 PSUM accumulation,
fp32 residual/LN/softmax paths):
  hT  [C=6x128, tok]   <- LN1 in token-major + PE transpose
  qT/kT per head pair [128 (2 heads x 64), tok]
  V'  [tok, 12, 65]    (65th column = ones -> PV matmul yields row sums)
  S^T [keys=128, 2h x 2qb x 512] per (head-pair, key-tile); exp on ACT
  O^T [65, 1024] accumulated over key tiles; row 64 = softmax denominators
  proj/MLP in token-major with PE transposes after LN2.

LN gains/biases are folded into the weights host-side (exact).
"""

import numpy as np
import ml_dtypes

import concourse.bass as bass
import concourse.tile as tile
from concourse import bacc, mybir
from concourse.bass_utils import run_bass_kernel_spmd
from concourse.masks import make_identity

F32 = mybir.dt.float32
BF16 = mybir.dt.bfloat16
AF = mybir.ActivationFunctionType

B, N, C = 4, 2048, 768
H, DH = 12, 64
HID = 4 * C
EPS = 1e-5
NCORES = 8

SEQ = N            # tokens per core's sequence (full, for K/V)
OWN = N // 2       # own query tokens per core
T16 = SEQ // 128   # token tiles (full seq)
T8 = OWN // 128    # token tiles (own)
CC = C // 128      # C chunks (6)
HP = H // 2        # head pairs (6)
HC = HID // 128    # hidden chunks (24)

_CACHE = {}


def _build_program(has_bqk: bool, has_bias2: bool, has_b2: bool,
                   debug_dumps: bool = False):
    nc = bacc.Bacc("TRN2", target_bir_lowering=False, debug=False,
                   num_devices=NCORES)
    dbg = {}
    if debug_dumps:
        dbg["hT"] = nc.dram_tensor("d_hT", [CC, 128, SEQ], F32,
                                   kind="ExternalOutput").ap()
        dbg["kT0"] = nc.dram_tensor("d_kT0", [128, SEQ], F32,
                                    kind="ExternalOutput").ap()
        dbg["qT0"] = nc.dram_tensor("d_qT0", [128, OWN], F32,
                                    kind="ExternalOutput").ap()
        dbg["v0"] = nc.dram_tensor("d_v0", [128, H, DH + 1], F32,
                                   kind="ExternalOutput").ap()
        dbg["pt00"] = nc.dram_tensor("d_pt00", [128, 1024], F32,
                                     kind="ExternalOutput").ap()
        dbg["oT0"] = nc.dram_tensor("d_oT0", [128, OWN], F32,
                                    kind="ExternalOutput").ap()
        dbg["y0"] = nc.dram_tensor("d_y0", [128, C], F32,
                                   kind="ExternalOutput").ap()
        dbg["h2T0"] = nc.dram_tensor("d_h2T0", [128, OWN], F32,
                                     kind="ExternalOutput").ap()
        dbg["m0"] = nc.dram_tensor("d_m0", [128, OWN], F32,
                                   kind="ExternalOutput").ap()

    # ---------------- DRAM I/O ----------------
    x_d = nc.dram_tensor("xseq", [OWN, C], F32, kind="ExternalInput").ap()
    wq_d = nc.dram_tensor("wq", [C, C], BF16, kind="ExternalInput").ap()
    wk_d = nc.dram_tensor("wk", [C, C], BF16, kind="ExternalInput").ap()
    wv_d = nc.dram_tensor("wv", [C, C], BF16, kind="ExternalInput").ap()
    wp_d = nc.dram_tensor("wproj", [C, C], BF16, kind="ExternalInput").ap()
    w1_d = nc.dram_tensor("w1", [C, HID], BF16, kind="ExternalInput").ap()
    w2_d = nc.dram_tensor("w2", [HID, C], BF16, kind="ExternalInput").ap()
    bq_d = nc.dram_tensor("bq", [C], F32, kind="ExternalInput").ap()
    bk_d = nc.dram_tensor("bk", [C], F32, kind="ExternalInput").ap()
    b1_d = nc.dram_tensor("b1e", [HID], F32, kind="ExternalInput").ap()
    bias2_d = nc.dram_tensor("bias2", [C], F32, kind="ExternalInput").ap()
    b2_d = nc.dram_tensor("b2t", [C], F32, kind="ExternalInput").ap()
    out_d = nc.dram_tensor("out", [OWN, C], F32, kind="ExternalOutput").ap()

    def bcast_row(dram_ap, parts=128):
        # DRAM [n] -> SBUF [parts, n] partition-broadcast AP
        return bass.AP(tensor=dram_ap.tensor, offset=dram_ap.offset,
                       ap=[[0, parts]] + list(dram_ap.ap))

    with tile.TileContext(nc) as tc:
        # ---------------- persistent constants ----------------
        consts = tc.alloc_tile_pool(name="consts", bufs=1)
        ident = consts.tile([128, 128], F32, tag="ident")
        make_identity(nc, ident)
        eps_t = consts.tile([128, 1], F32, tag="eps")
        nc.vector.memset(eps_t, EPS)
        bq_sb = consts.tile([128, CC], F32, tag="bq")
        nc.gpsimd.dma_start(out=bq_sb, in_=bq_d.rearrange("(a p) -> p a", p=128))
        bk_sb = consts.tile([128, CC], F32, tag="bk")
        nc.gpsimd.dma_start(out=bk_sb, in_=bk_d.rearrange("(a p) -> p a", p=128))
        b1_sb = consts.tile([128, HC], F32, tag="b1")
        nc.gpsimd.dma_start(out=b1_sb, in_=b1_d.rearrange("(a p) -> p a", p=128))
        if has_bias2:
            bias2_sb = consts.tile([128, C], F32, tag="bias2")
            nc.gpsimd.dma_start(out=bias2_sb, in_=bcast_row(bias2_d))
        if has_b2:
            b2_sb = consts.tile([128, C], F32, tag="b2")
            nc.gpsimd.dma_start(out=b2_sb, in_=bcast_row(b2_d))

        # persistent activation pools (phase-crossing lifetimes)
        # SBUF pool stacks must pop LIFO per side; long-lived pools are
        # ordered so each side's release order nests properly.
        xres_pool = tc.alloc_tile_pool(name="xres", bufs=1)
        xres = [xres_pool.tile([128, C], F32, tag=f"xr{t}", name=f"xr{t}") for t in range(T8)]
        hT_pool = tc.alloc_tile_pool(name="hT", bufs=1)
        hT = [hT_pool.tile([128, OWN], BF16, tag=f"hT{c}", name=f"hT{c}") for c in range(CC)]

        # weight prefetch: issue QKV weight DMAs before P1 so they
        # overlap the x loads / LN phase (pool sits above hT on the left
        # stack; released at end of P2).
        wqkv_pool = tc.alloc_tile_pool(name="wqkv", bufs=1)
        wq_sb = [wqkv_pool.tile([128, C], BF16, tag=f"wq{c}", name=f"wq{c}") for c in range(CC)]
        wk_sb = [wqkv_pool.tile([128, C], BF16, tag=f"wk{c}", name=f"wk{c}") for c in range(CC)]
        wv_sb = [wqkv_pool.tile([128, C], BF16, tag=f"wv{c}", name=f"wv{c}") for c in range(CC)]
        for c in range(CC):
            nc.sync.dma_start(out=wq_sb[c], in_=wq_d[c * 128:(c + 1) * 128, :])
            nc.sync.dma_start(out=wk_sb[c], in_=wk_d[c * 128:(c + 1) * 128, :])
            nc.sync.dma_start(out=wv_sb[c], in_=wv_d[c * 128:(c + 1) * 128, :])


        # ============ P1: LN1 + transpose -> hT ============
        p1x_pool = tc.alloc_tile_pool(name="p1x", bufs=3)
        p1_ps = tc.alloc_tile_pool(name="p1ps", bufs=4, space="PSUM")
        p1_sb = tc.alloc_tile_pool(name="p1sb", bufs=3)
        for t in range(T8):
            x_t = xres[t]
            nc.sync.dma_start(out=x_t, in_=x_d[t * 128:(t + 1) * 128, :])
            st = p1_sb.tile([128, 3, 6], F32, tag="stats")
            xg = x_t.rearrange("p (n s) -> p n s", s=256)
            for i in range(3):
                nc.vector.bn_stats(out=st[:, i, :], in_=xg[:, i, :])
            mv = p1_sb.tile([128, 2], F32, tag="mv")
            nc.vector.bn_aggr(out=mv, in_=st)
            rstd = p1_sb.tile([128, 1], F32, tag="rstd")
            nc.scalar.activation(out=rstd, in_=mv[:, 1:2], func=AF.Sqrt,
                                 bias=eps_t)
            nc.vector.reciprocal(out=rstd, in_=rstd)
            h_t = p1_sb.tile([128, C], F32, tag="h")
            nc.vector.tensor_scalar(out=h_t, in0=x_t, scalar1=mv[:, 0:1],
                                    scalar2=rstd,
                                    op0=mybir.AluOpType.subtract,
                                    op1=mybir.AluOpType.mult)
            for c in range(CC):
                tp = p1_ps.tile([128, 128], F32, tag="tp")
                nc.tensor.transpose(tp, h_t[:, c * 128:(c + 1) * 128], ident)
                nc.scalar.copy(out=hT[c][:, t * 128:(t + 1) * 128], in_=tp)
        p1_sb.release()
        p1_ps.release()
        if debug_dumps:
            for c in range(CC):
                nc.gpsimd.dma_start(out=dbg["hT"][c], in_=hT[c])

        # ============ P2: QKV ============
        kv_pool = tc.alloc_tile_pool(name="kv", bufs=1, side="right")
        kT = [kv_pool.tile([128, SEQ], BF16, tag=f"kT{p}", name=f"kT{p}") for p in range(HP)]
        qT = [kv_pool.tile([128, OWN], BF16, tag=f"qT{p}", name=f"qT{p}") for p in range(HP)]
        vP = [kv_pool.tile([128, H, DH + 1], BF16, tag=f"v{t}", name=f"v{t}")
              for t in range(T16)]

        # --- kT / qT / V' for OWN tokens; pair AllGather fills the rest ---
        cc_pool = tc.alloc_tile_pool(name="ccdram", bufs=1, space="DRAM")
        kin = cc_pool.tile([HP, 128, OWN], BF16, tag="kin", name="kin")
        kout = cc_pool.tile([2, HP, 128, OWN], BF16, tag="kout", name="kout")
        vin = cc_pool.tile([T8, 128, H * (DH + 1)], BF16, tag="vin", name="vin")
        vout = cc_pool.tile([2, T8, 128, H * (DH + 1)], BF16, tag="vout",
                            name="vout")
        GROUPS = [[0, 1], [2, 3], [4, 5], [6, 7]]

        p2k_ps = tc.alloc_tile_pool(name="p2kps", bufs=2, space="PSUM")
        p2q_ps = tc.alloc_tile_pool(name="p2qps", bufs=2, space="PSUM")
        for p in range(HP):
            kps = p2k_ps.tile([128, OWN], F32, tag="kps")
            for c in range(CC):
                for nb in range(OWN // 512):
                    nc.tensor.matmul(kps[:, nb * 512:(nb + 1) * 512],
                                     wk_sb[c][:, p * 128:(p + 1) * 128],
                                     hT[c][:, nb * 512:(nb + 1) * 512],
                                     start=(c == 0), stop=(c == CC - 1))
            kh = p1x_pool.tile([128, OWN], BF16, tag="kh", name="kh")
            if has_bqk:
                nc.vector.tensor_scalar(out=kh, in0=kps,
                                        scalar1=bk_sb[:, p:p + 1],
                                        op0=mybir.AluOpType.add)
            else:
                nc.scalar.copy(out=kh, in_=kps)
            nc.sync.dma_start(out=kin[p], in_=kh)

            qps = p2q_ps.tile([128, OWN], F32, tag="qps")
            for c in range(CC):
                for nb in range(OWN // 512):
                    nc.tensor.matmul(qps[:, nb * 512:(nb + 1) * 512],
                                     wq_sb[c][:, p * 128:(p + 1) * 128],
                                     hT[c][:, nb * 512:(nb + 1) * 512],
                                     start=(c == 0), stop=(c == CC - 1))
            if has_bqk:
                nc.vector.tensor_scalar(out=qT[p], in0=qps,
                                        scalar1=bq_sb[:, p:p + 1],
                                        op0=mybir.AluOpType.add)
            else:
                nc.scalar.copy(out=qT[p], in_=qps)
        p2q_ps.release()
        p2k_ps.release()

        p2v_ps = tc.alloc_tile_pool(name="p2vps", bufs=2, space="PSUM")
        for t in range(T8):
            vps = p2v_ps.tile([128, 1024], F32, tag="vps")
            for c in range(CC):
                nc.tensor.matmul(vps[:, 0:512],
                                 hT[c][:, t * 128:(t + 1) * 128],
                                 wv_sb[c][:, 0:512],
                                 start=(c == 0), stop=(c == CC - 1))
                nc.tensor.matmul(vps[:, 512:768],
                                 hT[c][:, t * 128:(t + 1) * 128],
                                 wv_sb[c][:, 512:768],
                                 start=(c == 0), stop=(c == CC - 1))
            vh = p1x_pool.tile([128, H, DH + 1], BF16, tag="vh", name="vh")
            nc.vector.tensor_copy(
                out=vh[:, :, 0:DH],
                in_=vps[:, 0:C].rearrange("p (g d) -> p g d", d=DH))
            nc.vector.memset(vh[:, :, DH:DH + 1], 1.0)
            nc.sync.dma_start(out=vin[t], in_=vh.rearrange("p g d -> p (g d)"))
        p2v_ps.release()
        p1x_pool.release()
        wqkv_pool.release()
        hT_pool.release()

        # pair-wise AllGather of K and V halves
        nc.gpsimd.collective_compute(
            "AllGather", mybir.AluOpType.bypass, replica_groups=GROUPS,
            ins=[kin[:]], outs=[kout[:]])
        nc.gpsimd.collective_compute(
            "AllGather", mybir.AluOpType.bypass, replica_groups=GROUPS,
            ins=[vin[:]], outs=[vout[:]])
        for p in range(HP):
            nc.sync.dma_start(out=kT[p][:, 0:OWN], in_=kout[0, p])
            nc.sync.dma_start(out=kT[p][:, OWN:SEQ], in_=kout[1, p])
        for t in range(T16):
            half, tt = divmod(t, T8)
            nc.sync.dma_start(
                out=vP[t].rearrange("p g d -> p (g d)"),
                in_=vout[half, tt])

        # ============ P3: attention ============
        oT_pool = tc.alloc_tile_pool(name="oT", bufs=1)
        oT = [oT_pool.tile([128, OWN], BF16, tag=f"oT{p}", name=f"oT{p}") for p in range(HP)]

        # prefetch proj weights during attention (DMA idle there)
        wp_pool = tc.alloc_tile_pool(name="wp", bufs=1)
        wp_sb = [wp_pool.tile([128, C], BF16, tag=f"wp{p}", name=f"wp{p}") for p in range(HP)]
        for p in range(HP):
            nc.sync.dma_start(out=wp_sb[p], in_=wp_d[p * 128:(p + 1) * 128, :])

        s_ps = tc.alloc_tile_pool(name="sps", bufs=2, space="PSUM")
        o_ps = tc.alloc_tile_pool(name="ops", bufs=1, space="PSUM")
        pt_pool = tc.alloc_tile_pool(name="pt", bufs=4)
        sm_pool = tc.alloc_tile_pool(name="sm", bufs=2)

        for p in range(HP):
            ops = [o_ps.tile([65, OWN], F32, tag=f"o{h}", name=f"o{h}") for h in range(2)]
            for kt in range(T16):
                for h in range(2):
                    sps = s_ps.tile([128, 1024], F32, tag="s", name="sps")
                    for qb in range(OWN // 512):
                        nc.tensor.matmul(
                            sps[:, qb * 512:(qb + 1) * 512],
                            kT[p][h * 64:(h + 1) * 64,
                                  kt * 128:(kt + 1) * 128],
                            qT[p][h * 64:(h + 1) * 64,
                                  qb * 512:(qb + 1) * 512],
                            start=True, stop=True)
                    ptt = pt_pool.tile([128, 1024], BF16, tag="pt", name="ptt")
                    nc.scalar.activation(out=ptt, in_=sps, func=AF.Exp,
                                         scale=float(DH) ** -0.5)
                    if debug_dumps and p == 0 and kt == 0 and h == 0:
                        nc.gpsimd.dma_start(out=dbg["pt00"], in_=ptt)
                    g = p * 2 + h
                    for qb in range(OWN // 512):
                        nc.tensor.matmul(
                            ops[h][:, qb * 512:(qb + 1) * 512],
                            vP[kt][:, g, :],
                            ptt[:, qb * 512:(qb + 1) * 512],
                            start=(kt == 0), stop=(kt == T16 - 1))
            # softmax normalization: O[0:64] * (1 / O[64]).
            # Copy O off PSUM immediately (frees the accumulator slot for
            # the next head pair); recip + partition-broadcast + multiply
            # then run on DVE/GPSIMD/DMA fully overlapped with the next
            # head pair's matmuls and exp.
            for h in range(2):
                ofull = sm_pool.tile([65, OWN], F32, tag="ofl", name="ofl")
                nc.vector.tensor_copy(out=ofull, in_=ops[h][0:65, :])
                rec = sm_pool.tile([65, OWN], F32, tag="rec", name="rec")
                nc.vector.reciprocal_approx_fast(rec, ofull)
                rrow = sm_pool.tile([1, OWN], F32, tag="rrow", name="rrow")
                nc.sync.dma_start(out=rrow, in_=rec[64:65, :])
                bcast = sm_pool.tile([64, OWN], F32, tag="bcast", name="bcast")
                nc.gpsimd.partition_broadcast(bcast, rrow[0:1, :])
                if h == 0:
                    nc.vector.tensor_mul(oT[p][0:64, :], ofull[0:64, :], bcast)
                else:
                    tmp = sm_pool.tile([64, OWN], BF16, tag="otmp", name="otmp")
                    nc.vector.tensor_mul(tmp, ofull[0:64, :], bcast)
                    nc.sync.dma_start(out=oT[p][64:128, :], in_=tmp)
        o_ps.release()
        s_ps.release()
        sm_pool.release()
        pt_pool.release()
        kv_pool.release()

        # ============ P4: proj + residual + LN2 + h2T ============
        y_pool = tc.alloc_tile_pool(name="y", bufs=1, side="right")
        y = [y_pool.tile([128, C], F32, tag=f"y{t}", name=f"y{t}") for t in range(T8)]
        h2T_pool = tc.alloc_tile_pool(name="h2T", bufs=1, side="right")
        h2T = [h2T_pool.tile([128, OWN], BF16, tag=f"h2T{c}", name=f"h2T{c}") for c in range(CC)]

        p4_ps = tc.alloc_tile_pool(name="p4ps", bufs=2, space="PSUM")
        p4t_ps = tc.alloc_tile_pool(name="p4tps", bufs=4, space="PSUM")
        p4_sb = tc.alloc_tile_pool(name="p4sb", bufs=3)
        for t in range(T8):
            aps = p4_ps.tile([128, 1024], F32, tag="aps")
            for p in range(HP):
                nc.tensor.matmul(aps[:, 0:512],
                                 oT[p][:, t * 128:(t + 1) * 128],
                                 wp_sb[p][:, 0:512],
                                 start=(p == 0), stop=(p == HP - 1))
                nc.tensor.matmul(aps[:, 512:768],
                                 oT[p][:, t * 128:(t + 1) * 128],
                                 wp_sb[p][:, 512:768],
                                 start=(p == 0), stop=(p == HP - 1))
            nc.vector.tensor_add(y[t], xres[t], aps[:, 0:C])
            if has_bias2:
                nc.vector.tensor_add(y[t], y[t], bias2_sb)
            st = p4_sb.tile([128, 3, 6], F32, tag="stats")
            yg = y[t].rearrange("p (n s) -> p n s", s=256)
            for i in range(3):
                nc.vector.bn_stats(out=st[:, i, :], in_=yg[:, i, :])
            mv = p4_sb.tile([128, 2], F32, tag="mv")
            nc.vector.bn_aggr(out=mv, in_=st)
            rstd = p4_sb.tile([128, 1], F32, tag="rstd")
            nc.scalar.activation(out=rstd, in_=mv[:, 1:2], func=AF.Sqrt,
                                 bias=eps_t)
            nc.vector.reciprocal(out=rstd, in_=rstd)
            h2 = p4_sb.tile([128, C], F32, tag="h2")
            nc.vector.tensor_scalar(out=h2, in0=y[t], scalar1=mv[:, 0:1],
                                    scalar2=rstd,
                                    op0=mybir.AluOpType.subtract,
                                    op1=mybir.AluOpType.mult)
            for c in range(CC):
                tp = p4t_ps.tile([128, 128], F32, tag="tp")
                nc.tensor.transpose(tp, h2[:, c * 128:(c + 1) * 128], ident)
                nc.scalar.copy(out=h2T[c][:, t * 128:(t + 1) * 128], in_=tp)
        if debug_dumps:
            nc.gpsimd.dma_start(out=dbg["y0"], in_=y[0])
            nc.gpsimd.dma_start(out=dbg["h2T0"], in_=h2T[0])
        p4_sb.release()
        p4t_ps.release()
        p4_ps.release()
        wp_pool.release()
        oT_pool.release()
        xres_pool.release()

        # ============ P5: MLP up + gelu ============
        m_pool = tc.alloc_tile_pool(name="m", bufs=1)
        mT = [m_pool.tile([128, OWN], BF16, tag=f"m{i}", name=f"m{i}") for i in range(HC)]
        w2_pool = tc.alloc_tile_pool(name="w2p", bufs=1)
        w2_sb = [w2_pool.tile([128, C], BF16, tag=f"w2{i}", name=f"w2{i}") for i in range(HC)]
        for i in range(HC):
            nc.sync.dma_start(out=w2_sb[i], in_=w2_d[i * 128:(i + 1) * 128, :])

        w1_pool = tc.alloc_tile_pool(name="w1p", bufs=1)
        w1_sb = [w1_pool.tile([128, HID], BF16, tag=f"w1{c}", name=f"w1{c}") for c in range(CC)]
        for c in range(CC):
            nc.sync.dma_start(out=w1_sb[c], in_=w1_d[c * 128:(c + 1) * 128, :])

        p5_ps = tc.alloc_tile_pool(name="p5ps", bufs=3, space="PSUM")
        for i in range(HC):
            hps = p5_ps.tile([128, 1024], F32, tag="hps")
            for c in range(CC):
                for qb in range(OWN // 512):
                    nc.tensor.matmul(hps[:, qb * 512:(qb + 1) * 512],
                                     w1_sb[c][:, i * 128:(i + 1) * 128],
                                     h2T[c][:, qb * 512:(qb + 1) * 512],
                                     start=(c == 0), stop=(c == CC - 1))
            nc.scalar.activation(out=mT[i], in_=hps, func=AF.Gelu,
                                 bias=b1_sb[:, i:i + 1])
        if debug_dumps:
            nc.gpsimd.dma_start(out=dbg["m0"], in_=mT[0])
        p5_ps.release()
        w1_pool.release()
        h2T_pool.release()

        # ============ P6: MLP down + final residual ============
        p6_ps = tc.alloc_tile_pool(name="p6ps", bufs=2, space="PSUM")
        p6_sb = tc.alloc_tile_pool(name="p6sb", bufs=3)
        for t in range(T8):
            mps = p6_ps.tile([128, 1024], F32, tag="mps")
            for i in range(HC):
                nc.tensor.matmul(mps[:, 0:512],
                                 mT[i][:, t * 128:(t + 1) * 128],
                                 w2_sb[i][:, 0:512],
                                 start=(i == 0), stop=(i == HC - 1))
                nc.tensor.matmul(mps[:, 512:768],
                                 mT[i][:, t * 128:(t + 1) * 128],
                                 w2_sb[i][:, 512:768],
                                 start=(i == 0), stop=(i == HC - 1))
            o_t = p6_sb.tile([128, C], F32, tag="out")
            nc.vector.tensor_add(o_t, y[t], mps[:, 0:C])
            if has_b2:
                nc.vector.tensor_add(o_t, o_t, b2_sb)
            nc.sync.dma_start(out=out_d[t * 128:(t + 1) * 128, :], in_=o_t)
        p6_ps.release()
        p6_sb.release()
        w2_pool.release()
        m_pool.release()
        y_pool.release()
        consts.release()

    nc.compile()
    return nc


def build_in_maps(x, ln1_g, ln1_b, w_qkv, w_proj, b_proj, ln2_g, ln2_b,
                  w1, b1, w2, b2):
    x = np.asarray(x, np.float32)
    ln1_g = np.asarray(ln1_g, np.float32)
    ln1_b = np.asarray(ln1_b, np.float32)
    w_qkv = np.asarray(w_qkv, np.float32)
    w_proj = np.asarray(w_proj, np.float32)
    b_proj = np.asarray(b_proj, np.float32)
    ln2_g = np.asarray(ln2_g, np.float32)
    ln2_b = np.asarray(ln2_b, np.float32)
    w1 = np.asarray(w1, np.float32)
    b1 = np.asarray(b1, np.float32)
    w2 = np.asarray(w2, np.float32)
    b2 = np.asarray(b2, np.float32)

    bf = ml_dtypes.bfloat16
    wqkv_eff = w_qkv * ln1_g[:, None]
    bqkv = ln1_b @ w_qkv
    wq = np.ascontiguousarray(wqkv_eff[:, 0:C]).astype(bf)
    wk = np.ascontiguousarray(wqkv_eff[:, C:2 * C]).astype(bf)
    wv = np.ascontiguousarray(wqkv_eff[:, 2 * C:3 * C]).astype(bf)
    bq = np.ascontiguousarray(bqkv[0:C])
    bk = np.ascontiguousarray(bqkv[C:2 * C])
    bv = np.ascontiguousarray(bqkv[2 * C:3 * C])
    bias2 = bv @ w_proj + b_proj
    w1_eff = (w1 * ln2_g[:, None]).astype(bf)
    b1_eff = b1 + ln2_b @ w1
    wp = w_proj.astype(bf)
    w2b = w2.astype(bf)

    has_bqk = bool(np.any(bq != 0) or np.any(bk != 0))
    has_bias2 = bool(np.any(bias2 != 0))
    has_b2 = bool(np.any(b2 != 0))

    common = {
        "wq": wq, "wk": wk, "wv": wv, "wproj": wp,
        "w1": w1_eff, "w2": w2b,
        "bq": bq, "bk": bk, "b1e": b1_eff,
        "bias2": bias2.astype(np.float32), "b2t": b2,
    }
    in_maps = []
    for c in range(NCORES):
        b, half = divmod(c, 2)
        m = dict(common)
        m["xseq"] = np.ascontiguousarray(x[b][half * OWN:(half + 1) * OWN])
        in_maps.append(m)
    return in_maps, (has_bqk, has_bias2, has_b2)


def kernel(**inputs):
    in_maps, key = build_in_maps(**inputs)
    if key not in _CACHE:
        _CACHE[key] = _build_program(*key)
    nc = _CACHE[key]
    res = run_bass_kernel_spmd(nc, in_maps, core_ids=list(range(NCORES)))
    out = np.empty((B, N, C), np.float32)
    for c in range(NCORES):
        b, half = divmod(c, 2)
        out[b, half * OWN:(half + 1) * OWN, :] = res.results[c]["out"]
    return out


# revision 20
# speedup vs baseline: 193.9667x; 193.9667x over previous
"""Trainium2 Bass kernel for a ViT-style transformer block (nn_Block).

Reference computation (per batch sequence):
    h   = LN(x) * g1 + b1
    qkv = h @ w_qkv ; attention (12 heads, dh=64, softmax over keys)
    x   = x + (attn_out @ w_proj + b_proj)
    h2  = LN(x) * g2 + b2
    out = x + gelu(h2 @ w1 + b1) @ w2 + b2

Sharding: 8 cores; core c handles half of sequence b=c//2 (1024 query
tokens).  The input sequence is host-rolled so each core's own tokens are
always rows [0:1024] (keeps the SPMD program identical across cores).
K/V are computed on-device for the full 2048-token sequence (redundantly
per core pair) so no collectives are needed.

Layout strategy (all matmul operands bf16, fp32 PSUM accumulation,
fp32 residual/LN/softmax paths):
  hT  [C=6x128, tok]   <- LN1 in token-major + PE transpose
  qT/kT per head pair [128 (2 heads x 64), tok]
  V'  [tok, 12, 65]    (65th column = ones -> PV matmul yields row sums)
  S^T [keys=128, 2h x 2qb x 512] per (head-pair, key-tile); exp on ACT
  O^T [65, 1024] accumulated over key tiles; row 64 = softmax denominators
  proj/MLP in token-major with PE transposes after LN2.

LN gains/biases are folded into the weights host-side (exact).
"""

import numpy as np
import ml_dtypes

import concourse.bass as bass
import concourse.tile as tile
from concourse import bacc, mybir
from concourse.bass_utils import run_bass_kernel_spmd
from concourse.masks import make_identity

F32 = mybir.dt.float32
BF16 = mybir.dt.bfloat16
AF = mybir.ActivationFunctionType

B, N, C = 4, 2048, 768
H, DH = 12, 64
HID = 4 * C
EPS = 1e-5
NCORES = 8

SEQ = N            # tokens per core's sequence (full, for K/V)
OWN = N // 2       # own query tokens per core
T16 = SEQ // 128   # token tiles (full seq)
T8 = OWN // 128    # token tiles (own)
CC = C // 128      # C chunks (6)
HP = H // 2        # head pairs (6)
HC = HID // 128    # hidden chunks (24)

_CACHE = {}


def _build_program(has_bqk: bool, has_bias2: bool, has_b2: bool,
                   debug_dumps: bool = False):
    nc = bacc.Bacc("TRN2", target_bir_lowering=False, debug=False,
                   num_devices=NCORES)
    dbg = {}
    if debug_dumps:
        dbg["hT"] = nc.dram_tensor("d_hT", [CC, 128, SEQ], F32,
                                   kind="ExternalOutput").ap()
        dbg["kT0"] = nc.dram_tensor("d_kT0", [128, SEQ], F32,
                                    kind="ExternalOutput").ap()
        dbg["qT0"] = nc.dram_tensor("d_qT0", [128, OWN], F32,
                                    kind="ExternalOutput").ap()
        dbg["v0"] = nc.dram_tensor("d_v0", [128, H, DH + 1], F32,
                                   kind="ExternalOutput").ap()
        dbg["pt00"] = nc.dram_tensor("d_pt00", [128, 1024], F32,
                                     kind="ExternalOutput").ap()
        dbg["oT0"] = nc.dram_tensor("d_oT0", [128, OWN], F32,
                                    kind="ExternalOutput").ap()
        dbg["y0"] = nc.dram_tensor("d_y0", [128, C], F32,
                                   kind="ExternalOutput").ap()
        dbg["h2T0"] = nc.dram_tensor("d_h2T0", [128, OWN], F32,
                                     kind="ExternalOutput").ap()
        dbg["m0"] = nc.dram_tensor("d_m0", [128, OWN], F32,
                                   kind="ExternalOutput").ap()

    # ---------------- DRAM I/O ----------------
    x_d = nc.dram_tensor("xseq", [SEQ, C], F32, kind="ExternalInput").ap()
    wq_d = nc.dram_tensor("wq", [C, C], BF16, kind="ExternalInput").ap()
    wk_d = nc.dram_tensor("wk", [C, C], BF16, kind="ExternalInput").ap()
    wv_d = nc.dram_tensor("wv", [C, C], BF16, kind="ExternalInput").ap()
    wp_d = nc.dram_tensor("wproj", [C, C], BF16, kind="ExternalInput").ap()
    w1_d = nc.dram_tensor("w1", [C, HID], BF16, kind="ExternalInput").ap()
    w2_d = nc.dram_tensor("w2", [HID, C], BF16, kind="ExternalInput").ap()
    bq_d = nc.dram_tensor("bq", [C], F32, kind="ExternalInput").ap()
    bk_d = nc.dram_tensor("bk", [C], F32, kind="ExternalInput").ap()
    b1_d = nc.dram_tensor("b1e", [HID], F32, kind="ExternalInput").ap()
    bias2_d = nc.dram_tensor("bias2", [C], F32, kind="ExternalInput").ap()
    b2_d = nc.dram_tensor("b2t", [C], F32, kind="ExternalInput").ap()
    out_d = nc.dram_tensor("out", [OWN, C], F32, kind="ExternalOutput").ap()

    def bcast_row(dram_ap, parts=128):
        # DRAM [n] -> SBUF [parts, n] partition-broadcast AP
        return bass.AP(tensor=dram_ap.tensor, offset=dram_ap.offset,
                       ap=[[0, parts]] + list(dram_ap.ap))

    with tile.TileContext(nc) as tc:
        # ---------------- persistent constants ----------------
        consts = tc.alloc_tile_pool(name="consts", bufs=1)
        ident = consts.tile([128, 128], F32, tag="ident")
        make_identity(nc, ident)
        eps_t = consts.tile([128, 1], F32, tag="eps")
        nc.vector.memset(eps_t, EPS)
        bq_sb = consts.tile([128, CC], F32, tag="bq")
        nc.gpsimd.dma_start(out=bq_sb, in_=bq_d.rearrange("(a p) -> p a", p=128))
        bk_sb = consts.tile([128, CC], F32, tag="bk")
        nc.gpsimd.dma_start(out=bk_sb, in_=bk_d.rearrange("(a p) -> p a", p=128))
        b1_sb = consts.tile([128, HC], F32, tag="b1")
        nc.gpsimd.dma_start(out=b1_sb, in_=b1_d.rearrange("(a p) -> p a", p=128))
        if has_bias2:
            bias2_sb = consts.tile([128, C], F32, tag="bias2")
            nc.gpsimd.dma_start(out=bias2_sb, in_=bcast_row(bias2_d))
        if has_b2:
            b2_sb = consts.tile([128, C], F32, tag="b2")
            nc.gpsimd.dma_start(out=b2_sb, in_=bcast_row(b2_d))

        # persistent activation pools (phase-crossing lifetimes)
        # SBUF pool stacks must pop LIFO per side; long-lived pools are
        # ordered so each side's release order nests properly.
        xres_pool = tc.alloc_tile_pool(name="xres", bufs=1)
        xres = [xres_pool.tile([128, C], F32, tag=f"xr{t}", name=f"xr{t}") for t in range(T8)]
        hT_pool = tc.alloc_tile_pool(name="hT", bufs=1)
        hT = [hT_pool.tile([128, SEQ], BF16, tag=f"hT{c}", name=f"hT{c}") for c in range(CC)]

        # weight prefetch: issue QKV weight DMAs before P1 so they
        # overlap the x loads / LN phase (pool sits above hT on the left
        # stack; released at end of P2).
        wqkv_pool = tc.alloc_tile_pool(name="wqkv", bufs=1)
        wq_sb = [wqkv_pool.tile([128, C], BF16, tag=f"wq{c}", name=f"wq{c}") for c in range(CC)]
        wk_sb = [wqkv_pool.tile([128, C], BF16, tag=f"wk{c}", name=f"wk{c}") for c in range(CC)]
        wv_sb = [wqkv_pool.tile([128, C], BF16, tag=f"wv{c}", name=f"wv{c}") for c in range(CC)]
        for c in range(CC):
            nc.sync.dma_start(out=wq_sb[c], in_=wq_d[c * 128:(c + 1) * 128, :])
            nc.sync.dma_start(out=wk_sb[c], in_=wk_d[c * 128:(c + 1) * 128, :])
            nc.sync.dma_start(out=wv_sb[c], in_=wv_d[c * 128:(c + 1) * 128, :])


        # ============ P1: LN1 + transpose -> hT ============
        p1_ps = tc.alloc_tile_pool(name="p1ps", bufs=4, space="PSUM")
        p1_sb = tc.alloc_tile_pool(name="p1sb", bufs=3)
        for t in range(T16):
            if t < T8:
                x_t = xres[t]
            else:
                x_t = p1_sb.tile([128, C], F32, tag="xin")
            nc.sync.dma_start(out=x_t, in_=x_d[t * 128:(t + 1) * 128, :])
            st = p1_sb.tile([128, 3, 6], F32, tag="stats")
            xg = x_t.rearrange("p (n s) -> p n s", s=256)
            for i in range(3):
                nc.vector.bn_stats(out=st[:, i, :], in_=xg[:, i, :])
            mv = p1_sb.tile([128, 2], F32, tag="mv")
            nc.vector.bn_aggr(out=mv, in_=st)
            rstd = p1_sb.tile([128, 1], F32, tag="rstd")
            nc.scalar.activation(out=rstd, in_=mv[:, 1:2], func=AF.Sqrt,
                                 bias=eps_t)
            nc.vector.reciprocal(out=rstd, in_=rstd)
            h_t = p1_sb.tile([128, C], F32, tag="h")
            nc.vector.tensor_scalar(out=h_t, in0=x_t, scalar1=mv[:, 0:1],
                                    scalar2=rstd,
                                    op0=mybir.AluOpType.subtract,
                                    op1=mybir.AluOpType.mult)
            for c in range(CC):
                tp = p1_ps.tile([128, 128], F32, tag="tp")
                nc.tensor.transpose(tp, h_t[:, c * 128:(c + 1) * 128], ident)
                nc.scalar.copy(out=hT[c][:, t * 128:(t + 1) * 128], in_=tp)
        p1_sb.release()
        p1_ps.release()
        if debug_dumps:
            for c in range(CC):
                nc.gpsimd.dma_start(out=dbg["hT"][c], in_=hT[c])

        # ============ P2: QKV ============
        kv_pool = tc.alloc_tile_pool(name="kv", bufs=1, side="right")
        kT = [kv_pool.tile([128, SEQ], BF16, tag=f"kT{p}", name=f"kT{p}") for p in range(HP)]
        qT = [kv_pool.tile([128, OWN], BF16, tag=f"qT{p}", name=f"qT{p}") for p in range(HP)]
        vP = [kv_pool.tile([128, H, DH + 1], BF16, tag=f"v{t}", name=f"v{t}")
              for t in range(T16)]

        # --- kT (full sequence) ---
        p2k_ps = tc.alloc_tile_pool(name="p2kps", bufs=2, space="PSUM")
        for p in range(HP):
            kps = p2k_ps.tile([128, SEQ], F32, tag="kps")
            for c in range(CC):
                for nb in range(SEQ // 512):
                    nc.tensor.matmul(kps[:, nb * 512:(nb + 1) * 512],
                                     wk_sb[c][:, p * 128:(p + 1) * 128],
                                     hT[c][:, nb * 512:(nb + 1) * 512],
                                     start=(c == 0), stop=(c == CC - 1))
            if has_bqk:
                nc.vector.tensor_scalar(out=kT[p], in0=kps,
                                        scalar1=bk_sb[:, p:p + 1],
                                        op0=mybir.AluOpType.add)
            else:
                nc.scalar.copy(out=kT[p], in_=kps)
        p2k_ps.release()

        # --- qT (own tokens) + V' ---
        p2q_ps = tc.alloc_tile_pool(name="p2qps", bufs=2, space="PSUM")
        p2v_ps = tc.alloc_tile_pool(name="p2vps", bufs=2, space="PSUM")
        for p in range(HP):
            qps = p2q_ps.tile([128, OWN], F32, tag="qps")
            for c in range(CC):
                for nb in range(OWN // 512):
                    nc.tensor.matmul(qps[:, nb * 512:(nb + 1) * 512],
                                     wq_sb[c][:, p * 128:(p + 1) * 128],
                                     hT[c][:, nb * 512:(nb + 1) * 512],
                                     start=(c == 0), stop=(c == CC - 1))
            if has_bqk:
                nc.vector.tensor_scalar(out=qT[p], in0=qps,
                                        scalar1=bq_sb[:, p:p + 1],
                                        op0=mybir.AluOpType.add)
            else:
                nc.scalar.copy(out=qT[p], in_=qps)
        for t in range(T16):
            vps = p2v_ps.tile([128, 1024], F32, tag="vps")
            for c in range(CC):
                nc.tensor.matmul(vps[:, 0:512],
                                 hT[c][:, t * 128:(t + 1) * 128],
                                 wv_sb[c][:, 0:512],
                                 start=(c == 0), stop=(c == CC - 1))
                nc.tensor.matmul(vps[:, 512:768],
                                 hT[c][:, t * 128:(t + 1) * 128],
                                 wv_sb[c][:, 512:768],
                                 start=(c == 0), stop=(c == CC - 1))
            nc.vector.tensor_copy(
                out=vP[t][:, :, 0:DH],
                in_=vps[:, 0:C].rearrange("p (g d) -> p g d", d=DH))
            nc.vector.memset(vP[t][:, :, DH:DH + 1], 1.0)
            if debug_dumps and t == 0:
                nc.gpsimd.dma_start(out=dbg["v0"], in_=vP[0])
        p2v_ps.release()
        p2q_ps.release()
        wqkv_pool.release()
        hT_pool.release()

        if debug_dumps:
            nc.gpsimd.dma_start(out=dbg["kT0"], in_=kT[0])
            nc.gpsimd.dma_start(out=dbg["qT0"], in_=qT[0])
        # ============ P3: attention ============
        oT_pool = tc.alloc_tile_pool(name="oT", bufs=1)
        oT = [oT_pool.tile([128, OWN], BF16, tag=f"oT{p}", name=f"oT{p}") for p in range(HP)]

        # prefetch proj weights during attention (DMA idle there)
        wp_pool = tc.alloc_tile_pool(name="wp", bufs=1)
        wp_sb = [wp_pool.tile([128, C], BF16, tag=f"wp{p}", name=f"wp{p}") for p in range(HP)]
        for p in range(HP):
            nc.sync.dma_start(out=wp_sb[p], in_=wp_d[p * 128:(p + 1) * 128, :])

        s_ps = tc.alloc_tile_pool(name="sps", bufs=2, space="PSUM")
        o_ps = tc.alloc_tile_pool(name="ops", bufs=1, space="PSUM")
        pt_pool = tc.alloc_tile_pool(name="pt", bufs=4)
        sm_pool = tc.alloc_tile_pool(name="sm", bufs=2)

        for p in range(HP):
            ops = [o_ps.tile([65, OWN], F32, tag=f"o{h}", name=f"o{h}") for h in range(2)]
            for kt in range(T16):
                for h in range(2):
                    sps = s_ps.tile([128, 1024], F32, tag="s", name="sps")
                    for qb in range(OWN // 512):
                        nc.tensor.matmul(
                            sps[:, qb * 512:(qb + 1) * 512],
                            kT[p][h * 64:(h + 1) * 64,
                                  kt * 128:(kt + 1) * 128],
                            qT[p][h * 64:(h + 1) * 64,
                                  qb * 512:(qb + 1) * 512],
                            start=True, stop=True)
                    ptt = pt_pool.tile([128, 1024], BF16, tag="pt", name="ptt")
                    nc.scalar.activation(out=ptt, in_=sps, func=AF.Exp,
                                         scale=float(DH) ** -0.5)
                    if debug_dumps and p == 0 and kt == 0 and h == 0:
                        nc.gpsimd.dma_start(out=dbg["pt00"], in_=ptt)
                    g = p * 2 + h
                    for qb in range(OWN // 512):
                        nc.tensor.matmul(
                            ops[h][:, qb * 512:(qb + 1) * 512],
                            vP[kt][:, g, :],
                            ptt[:, qb * 512:(qb + 1) * 512],
                            start=(kt == 0), stop=(kt == T16 - 1))
            # softmax normalization: O[0:64] * (1 / O[64]).
            # Copy O off PSUM immediately (frees the accumulator slot for
            # the next head pair); recip + partition-broadcast + multiply
            # then run on DVE/GPSIMD/DMA fully overlapped with the next
            # head pair's matmuls and exp.
            for h in range(2):
                ofull = sm_pool.tile([65, OWN], F32, tag="ofl", name="ofl")
                nc.vector.tensor_copy(out=ofull, in_=ops[h][0:65, :])
                rec = sm_pool.tile([65, OWN], F32, tag="rec", name="rec")
                nc.vector.reciprocal_approx_fast(rec, ofull)
                rrow = sm_pool.tile([1, OWN], F32, tag="rrow", name="rrow")
                nc.sync.dma_start(out=rrow, in_=rec[64:65, :])
                bcast = sm_pool.tile([64, OWN], F32, tag="bcast", name="bcast")
                nc.gpsimd.partition_broadcast(bcast, rrow[0:1, :])
                if h == 0:
                    nc.vector.tensor_mul(oT[p][0:64, :], ofull[0:64, :], bcast)
                else:
                    tmp = sm_pool.tile([64, OWN], BF16, tag="otmp", name="otmp")
                    nc.vector.tensor_mul(tmp, ofull[0:64, :], bcast)
                    nc.sync.dma_start(out=oT[p][64:128, :], in_=tmp)
        o_ps.release()
        s_ps.release()
        sm_pool.release()
        pt_pool.release()
        kv_pool.release()

        # ============ P4: proj + residual + LN2 + h2T ============
        y_pool = tc.alloc_tile_pool(name="y", bufs=1, side="right")
        y = [y_pool.tile([128, C], F32, tag=f"y{t}", name=f"y{t}") for t in range(T8)]
        h2T_pool = tc.alloc_tile_pool(name="h2T", bufs=1, side="right")
        h2T = [h2T_pool.tile([128, OWN], BF16, tag=f"h2T{c}", name=f"h2T{c}") for c in range(CC)]

        p4_ps = tc.alloc_tile_pool(name="p4ps", bufs=2, space="PSUM")
        p4t_ps = tc.alloc_tile_pool(name="p4tps", bufs=4, space="PSUM")
        p4_sb = tc.alloc_tile_pool(name="p4sb", bufs=3)
        for t in range(T8):
            aps = p4_ps.tile([128, 1024], F32, tag="aps")
            for p in range(HP):
                nc.tensor.matmul(aps[:, 0:512],
                                 oT[p][:, t * 128:(t + 1) * 128],
                                 wp_sb[p][:, 0:512],
                                 start=(p == 0), stop=(p == HP - 1))
                nc.tensor.matmul(aps[:, 512:768],
                                 oT[p][:, t * 128:(t + 1) * 128],
                                 wp_sb[p][:, 512:768],
                                 start=(p == 0), stop=(p == HP - 1))
            nc.vector.tensor_add(y[t], xres[t], aps[:, 0:C])
            if has_bias2:
                nc.vector.tensor_add(y[t], y[t], bias2_sb)
            st = p4_sb.tile([128, 3, 6], F32, tag="stats")
            yg = y[t].rearrange("p (n s) -> p n s", s=256)
            for i in range(3):
                nc.vector.bn_stats(out=st[:, i, :], in_=yg[:, i, :])
            mv = p4_sb.tile([128, 2], F32, tag="mv")
            nc.vector.bn_aggr(out=mv, in_=st)
            rstd = p4_sb.tile([128, 1], F32, tag="rstd")
            nc.scalar.activation(out=rstd, in_=mv[:, 1:2], func=AF.Sqrt,
                                 bias=eps_t)
            nc.vector.reciprocal(out=rstd, in_=rstd)
            h2 = p4_sb.tile([128, C], F32, tag="h2")
            nc.vector.tensor_scalar(out=h2, in0=y[t], scalar1=mv[:, 0:1],
                                    scalar2=rstd,
                                    op0=mybir.AluOpType.subtract,
                                    op1=mybir.AluOpType.mult)
            for c in range(CC):
                tp = p4t_ps.tile([128, 128], F32, tag="tp")
                nc.tensor.transpose(tp, h2[:, c * 128:(c + 1) * 128], ident)
                nc.scalar.copy(out=h2T[c][:, t * 128:(t + 1) * 128], in_=tp)
        if debug_dumps:
            nc.gpsimd.dma_start(out=dbg["y0"], in_=y[0])
            nc.gpsimd.dma_start(out=dbg["h2T0"], in_=h2T[0])
        p4_sb.release()
        p4t_ps.release()
        p4_ps.release()
        wp_pool.release()
        oT_pool.release()
        xres_pool.release()

        # ============ P5: MLP up + gelu ============
        m_pool = tc.alloc_tile_pool(name="m", bufs=1)
        mT = [m_pool.tile([128, OWN], BF16, tag=f"m{i}", name=f"m{i}") for i in range(HC)]
        w2_pool = tc.alloc_tile_pool(name="w2p", bufs=1)
        w2_sb = [w2_pool.tile([128, C], BF16, tag=f"w2{i}", name=f"w2{i}") for i in range(HC)]
        for i in range(HC):
            nc.sync.dma_start(out=w2_sb[i], in_=w2_d[i * 128:(i + 1) * 128, :])

        w1_pool = tc.alloc_tile_pool(name="w1p", bufs=1)
        w1_sb = [w1_pool.tile([128, HID], BF16, tag=f"w1{c}", name=f"w1{c}") for c in range(CC)]
        for c in range(CC):
            nc.sync.dma_start(out=w1_sb[c], in_=w1_d[c * 128:(c + 1) * 128, :])

        p5_ps = tc.alloc_tile_pool(name="p5ps", bufs=3, space="PSUM")
        for i in range(HC):
            hps = p5_ps.tile([128, 1024], F32, tag="hps")
            for c in range(CC):
                for qb in range(OWN // 512):
                    nc.tensor.matmul(hps[:, qb * 512:(qb + 1) * 512],
                                     w1_sb[c][:, i * 128:(i + 1) * 128],
                                     h2T[c][:, qb * 512:(qb + 1) * 512],
                                     start=(c == 0), stop=(c == CC - 1))
            nc.scalar.activation(out=mT[i], in_=hps, func=AF.Gelu,
                                 bias=b1_sb[:, i:i + 1])
        if debug_dumps:
            nc.gpsimd.dma_start(out=dbg["m0"], in_=mT[0])
        p5_ps.release()
        w1_pool.release()
        h2T_pool.release()

        # ============ P6: MLP down + final residual ============
        p6_ps = tc.alloc_tile_pool(name="p6ps", bufs=2, space="PSUM")
        p6_sb = tc.alloc_tile_pool(name="p6sb", bufs=3)
        for t in range(T8):
            mps = p6_ps.tile([128, 1024], F32, tag="mps")
            for i in range(HC):
                nc.tensor.matmul(mps[:, 0:512],
                                 mT[i][:, t * 128:(t + 1) * 128],
                                 w2_sb[i][:, 0:512],
                                 start=(i == 0), stop=(i == HC - 1))
                nc.tensor.matmul(mps[:, 512:768],
                                 mT[i][:, t * 128:(t + 1) * 128],
                                 w2_sb[i][:, 512:768],
                                 start=(i == 0), stop=(i == HC - 1))
            o_t = p6_sb.tile([128, C], F32, tag="out")
            nc.vector.tensor_add(o_t, y[t], mps[:, 0:C])
            if has_b2:
                nc.vector.tensor_add(o_t, o_t, b2_sb)
            nc.sync.dma_start(out=out_d[t * 128:(t + 1) * 128, :], in_=o_t)
        p6_ps.release()
        p6_sb.release()
        w2_pool.release()
        m_pool.release()
        y_pool.release()
        consts.release()

    nc.compile()
    return nc


def build_in_maps(x, ln1_g, ln1_b, w_qkv, w_proj, b_proj, ln2_g, ln2_b,
                  w1, b1, w2, b2):
    x = np.asarray(x, np.float32)
    ln1_g = np.asarray(ln1_g, np.float32)
    ln1_b = np.asarray(ln1_b, np.float32)
    w_qkv = np.asarray(w_qkv, np.float32)
    w_proj = np.asarray(w_proj, np.float32)
    b_proj = np.asarray(b_proj, np.float32)
    ln2_g = np.asarray(ln2_g, np.float32)
    ln2_b = np.asarray(ln2_b, np.float32)
    w1 = np.asarray(w1, np.float32)
    b1 = np.asarray(b1, np.float32)
    w2 = np.asarray(w2, np.float32)
    b2 = np.asarray(b2, np.float32)

    bf = ml_dtypes.bfloat16
    wqkv_eff = w_qkv * ln1_g[:, None]
    bqkv = ln1_b @ w_qkv
    wq = np.ascontiguousarray(wqkv_eff[:, 0:C]).astype(bf)
    wk = np.ascontiguousarray(wqkv_eff[:, C:2 * C]).astype(bf)
    wv = np.ascontiguousarray(wqkv_eff[:, 2 * C:3 * C]).astype(bf)
    bq = np.ascontiguousarray(bqkv[0:C])
    bk = np.ascontiguousarray(bqkv[C:2 * C])
    bv = np.ascontiguousarray(bqkv[2 * C:3 * C])
    bias2 = bv @ w_proj + b_proj
    w1_eff = (w1 * ln2_g[:, None]).astype(bf)
    b1_eff = b1 + ln2_b @ w1
    wp = w_proj.astype(bf)
    w2b = w2.astype(bf)

    has_bqk = bool(np.any(bq != 0) or np.any(bk != 0))
    has_bias2 = bool(np.any(bias2 != 0))
    has_b2 = bool(np.any(b2 != 0))

    common = {
        "wq": wq, "wk": wk, "wv": wv, "wproj": wp,
        "w1": w1_eff, "w2": w2b,
        "bq": bq, "bk": bk, "b1e": b1_eff,
        "bias2": bias2.astype(np.float32), "b2t": b2,
    }
    in_maps = []
    for c in range(NCORES):
        b, half = divmod(c, 2)
        if half == 0:
            xseq = x[b]
        else:
            xseq = np.concatenate([x[b][OWN:], x[b][:OWN]], axis=0)
        m = dict(common)
        m["xseq"] = np.ascontiguousarray(xseq)
        in_maps.append(m)
    return in_maps, (has_bqk, has_bias2, has_b2)


def kernel(**inputs):
    in_maps, key = build_in_maps(**inputs)
    if key not in _CACHE:
        _CACHE[key] = _build_program(*key)
    nc = _CACHE[key]
    res = run_bass_kernel_spmd(nc, in_maps, core_ids=list(range(NCORES)))
    out = np.empty((B, N, C), np.float32)
    for c in range(NCORES):
        b, half = divmod(c, 2)
        out[b, half * OWN:(half + 1) * OWN, :] = res.results[c]["out"]
    return out


# revision 24
# speedup vs baseline: 199.0279x; 1.0261x over previous
"""Trainium2 Bass kernel for a ViT-style transformer block (nn_Block).

Reference computation (per batch sequence):
    h   = LN(x) * g1 + b1
    qkv = h @ w_qkv ; attention (12 heads, dh=64, softmax over keys)
    x   = x + (attn_out @ w_proj + b_proj)
    h2  = LN(x) * g2 + b2
    out = x + gelu(h2 @ w1 + b1) @ w2 + b2

Sharding: 8 cores; core c handles half of sequence b=c//2 (1024 query
tokens).  The input sequence is host-rolled so each core's own tokens are
always rows [0:1024] (keeps the SPMD program identical across cores).
K/V are computed on-device for the full 2048-token sequence (redundantly
per core pair) so no collectives are needed.

Layout strategy (all matmul operands bf16, fp32 PSUM accumulation,
fp32 residual/LN/softmax paths):
  hT  [C=6x128, tok]   <- LN1 in token-major + PE transpose
  qT/kT per head pair [128 (2 heads x 64), tok]
  V'  [tok, 12, 65]    (65th column = ones -> PV matmul yields row sums)
  S^T [keys=128, 2h x 2qb x 512] per (head-pair, key-tile); exp on ACT
  O^T [65, 1024] accumulated over key tiles; row 64 = softmax denominators
  proj/MLP in token-major with PE transposes after LN2.

LN gains/biases are folded into the weights host-side (exact).
"""

import numpy as np
import ml_dtypes

import concourse.bass as bass
import concourse.tile as tile
from concourse import bacc, mybir
from concourse.bass_utils import run_bass_kernel_spmd
from concourse.masks import make_identity

F32 = mybir.dt.float32
BF16 = mybir.dt.bfloat16
AF = mybir.ActivationFunctionType

B, N, C = 4, 2048, 768
H, DH = 12, 64
HID = 4 * C
EPS = 1e-5
NCORES = 8

SEQ = N            # tokens per core's sequence (full, for K/V)
OWN = N // 2       # own query tokens per core
T16 = SEQ // 128   # token tiles (full seq)
T8 = OWN // 128    # token tiles (own)
CC = C // 128      # C chunks (6)
HP = H // 2        # head pairs (6)
HC = HID // 128    # hidden chunks (24)

_CACHE = {}


def _build_program(has_bqk: bool, has_bias2: bool, has_b2: bool,
                   debug_dumps: bool = False):
    nc = bacc.Bacc("TRN2", target_bir_lowering=False, debug=False,
                   num_devices=NCORES)
    dbg = {}
    if debug_dumps:
        dbg["hT"] = nc.dram_tensor("d_hT", [CC, 128, SEQ], F32,
                                   kind="ExternalOutput").ap()
        dbg["kT0"] = nc.dram_tensor("d_kT0", [128, SEQ], F32,
                                    kind="ExternalOutput").ap()
        dbg["qT0"] = nc.dram_tensor("d_qT0", [128, OWN], F32,
                                    kind="ExternalOutput").ap()
        dbg["v0"] = nc.dram_tensor("d_v0", [128, H, DH + 1], F32,
                                   kind="ExternalOutput").ap()
        dbg["pt00"] = nc.dram_tensor("d_pt00", [128, 1024], F32,
                                     kind="ExternalOutput").ap()
        dbg["oT0"] = nc.dram_tensor("d_oT0", [128, OWN], F32,
                                    kind="ExternalOutput").ap()
        dbg["y0"] = nc.dram_tensor("d_y0", [128, C], F32,
                                   kind="ExternalOutput").ap()
        dbg["h2T0"] = nc.dram_tensor("d_h2T0", [128, OWN], F32,
                                     kind="ExternalOutput").ap()
        dbg["m0"] = nc.dram_tensor("d_m0", [128, OWN], F32,
                                   kind="ExternalOutput").ap()

    # ---------------- DRAM I/O ----------------
    x_d = nc.dram_tensor("xseq", [SEQ, C], F32, kind="ExternalInput").ap()
    wq_d = nc.dram_tensor("wq", [C, C], BF16, kind="ExternalInput").ap()
    wk_d = nc.dram_tensor("wk", [C, C], BF16, kind="ExternalInput").ap()
    wv_d = nc.dram_tensor("wv", [C, C], BF16, kind="ExternalInput").ap()
    wp_d = nc.dram_tensor("wproj", [C, C], BF16, kind="ExternalInput").ap()
    w1_d = nc.dram_tensor("w1", [C, HID], BF16, kind="ExternalInput").ap()
    w2_d = nc.dram_tensor("w2", [HID, C], BF16, kind="ExternalInput").ap()
    bq_d = nc.dram_tensor("bq", [C], F32, kind="ExternalInput").ap()
    bk_d = nc.dram_tensor("bk", [C], F32, kind="ExternalInput").ap()
    b1_d = nc.dram_tensor("b1e", [HID], F32, kind="ExternalInput").ap()
    bias2_d = nc.dram_tensor("bias2", [C], F32, kind="ExternalInput").ap()
    b2_d = nc.dram_tensor("b2t", [C], F32, kind="ExternalInput").ap()
    out_d = nc.dram_tensor("out", [OWN, C], F32, kind="ExternalOutput").ap()

    def bcast_row(dram_ap, parts=128):
        # DRAM [n] -> SBUF [parts, n] partition-broadcast AP
        return bass.AP(tensor=dram_ap.tensor, offset=dram_ap.offset,
                       ap=[[0, parts]] + list(dram_ap.ap))

    with tile.TileContext(nc) as tc:
        # ---------------- persistent constants ----------------
        consts = tc.alloc_tile_pool(name="consts", bufs=1)
        ident = consts.tile([128, 128], BF16, tag="ident")
        make_identity(nc, ident)
        eps_t = consts.tile([128, 1], F32, tag="eps")
        nc.vector.memset(eps_t, EPS)
        bq_sb = consts.tile([128, CC], F32, tag="bq")
        nc.gpsimd.dma_start(out=bq_sb, in_=bq_d.rearrange("(a p) -> p a", p=128))
        bk_sb = consts.tile([128, CC], F32, tag="bk")
        nc.gpsimd.dma_start(out=bk_sb, in_=bk_d.rearrange("(a p) -> p a", p=128))
        b1_sb = consts.tile([128, HC], F32, tag="b1")
        nc.gpsimd.dma_start(out=b1_sb, in_=b1_d.rearrange("(a p) -> p a", p=128))
        if has_bias2:
            bias2_sb = consts.tile([128, C], F32, tag="bias2")
            nc.gpsimd.dma_start(out=bias2_sb, in_=bcast_row(bias2_d))
        if has_b2:
            b2_sb = consts.tile([128, C], F32, tag="b2")
            nc.gpsimd.dma_start(out=b2_sb, in_=bcast_row(b2_d))

        # persistent activation pools (phase-crossing lifetimes)
        # SBUF pool stacks must pop LIFO per side; long-lived pools are
        # ordered so each side's release order nests properly.
        xres_pool = tc.alloc_tile_pool(name="xres", bufs=1)
        xres = [xres_pool.tile([128, C], F32, tag=f"xr{t}", name=f"xr{t}") for t in range(T8)]
        hT_pool = tc.alloc_tile_pool(name="hT", bufs=1)
        hT = [hT_pool.tile([128, SEQ], BF16, tag=f"hT{c}", name=f"hT{c}") for c in range(CC)]

        # weight prefetch: issue QKV weight DMAs before P1 so they
        # overlap the x loads / LN phase (pool sits above hT on the left
        # stack; released at end of P2).
        wqkv_pool = tc.alloc_tile_pool(name="wqkv", bufs=1)
        wq_sb = [wqkv_pool.tile([128, C], BF16, tag=f"wq{c}", name=f"wq{c}") for c in range(CC)]
        wk_sb = [wqkv_pool.tile([128, C], BF16, tag=f"wk{c}", name=f"wk{c}") for c in range(CC)]
        wv_sb = [wqkv_pool.tile([128, C], BF16, tag=f"wv{c}", name=f"wv{c}") for c in range(CC)]
        for c in range(CC):
            nc.sync.dma_start(out=wq_sb[c], in_=wq_d[c * 128:(c + 1) * 128, :])
            nc.sync.dma_start(out=wk_sb[c], in_=wk_d[c * 128:(c + 1) * 128, :])
            nc.sync.dma_start(out=wv_sb[c], in_=wv_d[c * 128:(c + 1) * 128, :])


        # ============ P1: LN1 + transpose -> hT ============
        p1_ps = tc.alloc_tile_pool(name="p1ps", bufs=4, space="PSUM")
        p1_sb = tc.alloc_tile_pool(name="p1sb", bufs=3)
        for t in range(T16):
            if t < T8:
                x_t = xres[t]
            else:
                x_t = p1_sb.tile([128, C], F32, tag="xin")
            nc.sync.dma_start(out=x_t, in_=x_d[t * 128:(t + 1) * 128, :])
            st = p1_sb.tile([128, 3, 6], F32, tag="stats")
            xg = x_t.rearrange("p (n s) -> p n s", s=256)
            for i in range(3):
                nc.vector.bn_stats(out=st[:, i, :], in_=xg[:, i, :])
            mv = p1_sb.tile([128, 2], F32, tag="mv")
            nc.vector.bn_aggr(out=mv, in_=st)
            rstd = p1_sb.tile([128, 1], F32, tag="rstd")
            nc.scalar.activation(out=rstd, in_=mv[:, 1:2], func=AF.Sqrt,
                                 bias=eps_t)
            nc.vector.reciprocal(out=rstd, in_=rstd)
            h_t = p1_sb.tile([128, C], BF16, tag="h")
            nc.vector.tensor_scalar(out=h_t, in0=x_t, scalar1=mv[:, 0:1],
                                    scalar2=rstd,
                                    op0=mybir.AluOpType.subtract,
                                    op1=mybir.AluOpType.mult)
            for c in range(CC):
                tp = p1_ps.tile([128, 128], BF16, tag="tp")
                nc.tensor.transpose(tp, h_t[:, c * 128:(c + 1) * 128], ident)
                nc.scalar.copy(out=hT[c][:, t * 128:(t + 1) * 128], in_=tp)
        p1_sb.release()
        p1_ps.release()
        if debug_dumps:
            for c in range(CC):
                nc.gpsimd.dma_start(out=dbg["hT"][c], in_=hT[c])

        # ============ P2: QKV ============
        kv_pool = tc.alloc_tile_pool(name="kv", bufs=1, side="right")
        kT = [kv_pool.tile([128, SEQ], BF16, tag=f"kT{p}", name=f"kT{p}") for p in range(HP)]
        qT = [kv_pool.tile([128, OWN], BF16, tag=f"qT{p}", name=f"qT{p}") for p in range(HP)]
        vP = [kv_pool.tile([128, H, DH + 1], BF16, tag=f"v{t}", name=f"v{t}")
              for t in range(T16)]

        # --- kT (full sequence) ---
        p2k_ps = tc.alloc_tile_pool(name="p2kps", bufs=2, space="PSUM")
        for p in range(HP):
            kps = p2k_ps.tile([128, SEQ], F32, tag="kps")
            for c in range(CC):
                for nb in range(SEQ // 512):
                    nc.tensor.matmul(kps[:, nb * 512:(nb + 1) * 512],
                                     wk_sb[c][:, p * 128:(p + 1) * 128],
                                     hT[c][:, nb * 512:(nb + 1) * 512],
                                     start=(c == 0), stop=(c == CC - 1))
            if has_bqk:
                nc.vector.tensor_scalar(out=kT[p], in0=kps,
                                        scalar1=bk_sb[:, p:p + 1],
                                        op0=mybir.AluOpType.add)
            else:
                nc.scalar.copy(out=kT[p], in_=kps)
        p2k_ps.release()

        # --- qT (own tokens) + V' ---
        p2q_ps = tc.alloc_tile_pool(name="p2qps", bufs=2, space="PSUM")
        for p in range(HP):
            qps = p2q_ps.tile([128, OWN], F32, tag="qps")
            for c in range(CC):
                for nb in range(OWN // 512):
                    nc.tensor.matmul(qps[:, nb * 512:(nb + 1) * 512],
                                     wq_sb[c][:, p * 128:(p + 1) * 128],
                                     hT[c][:, nb * 512:(nb + 1) * 512],
                                     start=(c == 0), stop=(c == CC - 1))
            if has_bqk:
                nc.vector.tensor_scalar(out=qT[p], in0=qps,
                                        scalar1=bq_sb[:, p:p + 1],
                                        op0=mybir.AluOpType.add)
            else:
                nc.scalar.copy(out=qT[p], in_=qps)
        p2q_ps.release()

        if debug_dumps:
            nc.gpsimd.dma_start(out=dbg["kT0"], in_=kT[0])
            nc.gpsimd.dma_start(out=dbg["qT0"], in_=qT[0])
        # ============ P3: attention ============
        oT_pool = tc.alloc_tile_pool(name="oT", bufs=1)
        oT = [oT_pool.tile([128, OWN], BF16, tag=f"oT{p}", name=f"oT{p}") for p in range(HP)]

        # prefetch proj weights during attention (DMA idle there)
        wp_pool = tc.alloc_tile_pool(name="wp", bufs=1)
        wp_sb = [wp_pool.tile([128, C], BF16, tag=f"wp{p}", name=f"wp{p}") for p in range(HP)]
        for p in range(HP):
            nc.sync.dma_start(out=wp_sb[p], in_=wp_d[p * 128:(p + 1) * 128, :])

        s_ps = tc.alloc_tile_pool(name="sps", bufs=2, space="PSUM")
        o_ps = tc.alloc_tile_pool(name="ops", bufs=1, space="PSUM")
        v_ps = tc.alloc_tile_pool(name="vps", bufs=1, space="PSUM")
        pt_pool = tc.alloc_tile_pool(name="pt", bufs=4)
        sm_pool = tc.alloc_tile_pool(name="sm", bufs=2)

        # V' production emitted here: its matmuls fill the PE slack while
        # the attention loop below keeps ACT saturated with exp ops.
        for t in range(T16):
            vps = v_ps.tile([128, 1024], F32, tag="vps")
            for c in range(CC):
                nc.tensor.matmul(vps[:, 0:512],
                                 hT[c][:, t * 128:(t + 1) * 128],
                                 wv_sb[c][:, 0:512],
                                 start=(c == 0), stop=(c == CC - 1))
                nc.tensor.matmul(vps[:, 512:768],
                                 hT[c][:, t * 128:(t + 1) * 128],
                                 wv_sb[c][:, 512:768],
                                 start=(c == 0), stop=(c == CC - 1))
            nc.vector.tensor_copy(
                out=vP[t][:, :, 0:DH],
                in_=vps[:, 0:C].rearrange("p (g d) -> p g d", d=DH))
            nc.vector.memset(vP[t][:, :, DH:DH + 1], 1.0)

        for p in range(HP):
            for qb in range(OWN // 512):
                ops = [o_ps.tile([65, 512], F32, tag=f"o{h}", name=f"o{h}")
                       for h in range(2)]
                for kt in range(T16):
                    sps = s_ps.tile([128, 1024], F32, tag="s", name="sps")
                    for h in range(2):
                        nc.tensor.matmul(
                            sps[:, h * 512:(h + 1) * 512],
                            kT[p][h * 64:(h + 1) * 64,
                                  kt * 128:(kt + 1) * 128],
                            qT[p][h * 64:(h + 1) * 64,
                                  qb * 512:(qb + 1) * 512],
                            start=True, stop=True)
                    ptt = pt_pool.tile([128, 1024], BF16, tag="pt", name="ptt")
                    nc.scalar.activation(out=ptt, in_=sps, func=AF.Exp,
                                         scale=float(DH) ** -0.5)
                    for h in range(2):
                        g = p * 2 + h
                        nc.tensor.matmul(
                            ops[h][:, 0:512],
                            vP[kt][:, g, :],
                            ptt[:, h * 512:(h + 1) * 512],
                            start=(kt == 0), stop=(kt == T16 - 1))
                # softmax normalization for this (head pair, query block):
                # copy O off PSUM immediately; recip + partition-broadcast +
                # multiply run on DVE/GPSIMD/DMA off the critical path.
                for h in range(2):
                    ofull = sm_pool.tile([65, 512], F32, tag="ofl", name="ofl")
                    nc.vector.tensor_copy(out=ofull, in_=ops[h][0:65, :])
                    rec = sm_pool.tile([65, 512], F32, tag="rec", name="rec")
                    nc.vector.reciprocal_approx_fast(rec, ofull)
                    rrow = sm_pool.tile([1, 512], F32, tag="rrow", name="rrow")
                    nc.sync.dma_start(out=rrow, in_=rec[64:65, :])
                    bcast = sm_pool.tile([64, 512], F32, tag="bcast",
                                         name="bcast")
                    nc.gpsimd.partition_broadcast(bcast, rrow[0:1, :])
                    if h == 0:
                        nc.vector.tensor_mul(
                            oT[p][0:64, qb * 512:(qb + 1) * 512],
                            ofull[0:64, :], bcast)
                    else:
                        tmp = sm_pool.tile([64, 512], BF16, tag="otmp",
                                           name="otmp")
                        nc.vector.tensor_mul(tmp, ofull[0:64, :], bcast)
                        nc.sync.dma_start(
                            out=oT[p][64:128, qb * 512:(qb + 1) * 512],
                            in_=tmp)
        v_ps.release()
        o_ps.release()
        s_ps.release()
        sm_pool.release()
        pt_pool.release()
        kv_pool.release()

        # ============ P4: proj + residual + LN2 + h2T ============
        y_pool = tc.alloc_tile_pool(name="y", bufs=1, side="right")
        y = [y_pool.tile([128, C], F32, tag=f"y{t}", name=f"y{t}") for t in range(T8)]
        h2T_pool = tc.alloc_tile_pool(name="h2T", bufs=1, side="right")
        h2T = [h2T_pool.tile([128, OWN], BF16, tag=f"h2T{c}", name=f"h2T{c}") for c in range(CC)]
        # prefetch MLP-up weights during proj/LN2 (right side: reuses the
        # K/Q/V space released at end of attention)
        w1_pool = tc.alloc_tile_pool(name="w1p", bufs=1, side="right")
        w1_sb = [w1_pool.tile([128, HID], BF16, tag=f"w1{c}", name=f"w1{c}") for c in range(CC)]
        for c in range(CC):
            nc.sync.dma_start(out=w1_sb[c], in_=w1_d[c * 128:(c + 1) * 128, :])

        p4_ps = tc.alloc_tile_pool(name="p4ps", bufs=2, space="PSUM")
        p4t_ps = tc.alloc_tile_pool(name="p4tps", bufs=4, space="PSUM")
        p4_sb = tc.alloc_tile_pool(name="p4sb", bufs=3)
        for t in range(T8):
            aps = p4_ps.tile([128, 1024], F32, tag="aps")
            for p in range(HP):
                nc.tensor.matmul(aps[:, 0:512],
                                 oT[p][:, t * 128:(t + 1) * 128],
                                 wp_sb[p][:, 0:512],
                                 start=(p == 0), stop=(p == HP - 1))
                nc.tensor.matmul(aps[:, 512:768],
                                 oT[p][:, t * 128:(t + 1) * 128],
                                 wp_sb[p][:, 512:768],
                                 start=(p == 0), stop=(p == HP - 1))
            nc.vector.tensor_add(y[t], xres[t], aps[:, 0:C])
            if has_bias2:
                nc.vector.tensor_add(y[t], y[t], bias2_sb)
            st = p4_sb.tile([128, 3, 6], F32, tag="stats")
            yg = y[t].rearrange("p (n s) -> p n s", s=256)
            for i in range(3):
                nc.vector.bn_stats(out=st[:, i, :], in_=yg[:, i, :])
            mv = p4_sb.tile([128, 2], F32, tag="mv")
            nc.vector.bn_aggr(out=mv, in_=st)
            rstd = p4_sb.tile([128, 1], F32, tag="rstd")
            nc.scalar.activation(out=rstd, in_=mv[:, 1:2], func=AF.Sqrt,
                                 bias=eps_t)
            nc.vector.reciprocal(out=rstd, in_=rstd)
            h2 = p4_sb.tile([128, C], BF16, tag="h2")
            nc.vector.tensor_scalar(out=h2, in0=y[t], scalar1=mv[:, 0:1],
                                    scalar2=rstd,
                                    op0=mybir.AluOpType.subtract,
                                    op1=mybir.AluOpType.mult)
            for c in range(CC):
                tp = p4t_ps.tile([128, 128], BF16, tag="tp")
                nc.tensor.transpose(tp, h2[:, c * 128:(c + 1) * 128], ident)
                nc.scalar.copy(out=h2T[c][:, t * 128:(t + 1) * 128], in_=tp)
        if debug_dumps:
            nc.gpsimd.dma_start(out=dbg["y0"], in_=y[0])
            nc.gpsimd.dma_start(out=dbg["h2T0"], in_=h2T[0])
        p4_sb.release()
        p4t_ps.release()
        p4_ps.release()
        wp_pool.release()
        oT_pool.release()
        wqkv_pool.release()
        hT_pool.release()
        xres_pool.release()

        # ============ P5: MLP up + gelu ============
        m_pool = tc.alloc_tile_pool(name="m", bufs=1)
        mT = [m_pool.tile([128, OWN], BF16, tag=f"m{i}", name=f"m{i}") for i in range(HC)]
        w2_pool = tc.alloc_tile_pool(name="w2p", bufs=1)
        w2_sb = [w2_pool.tile([128, C], BF16, tag=f"w2{i}", name=f"w2{i}") for i in range(HC)]
        for i in range(HC):
            nc.sync.dma_start(out=w2_sb[i], in_=w2_d[i * 128:(i + 1) * 128, :])


        p5_ps = tc.alloc_tile_pool(name="p5ps", bufs=3, space="PSUM")
        for i in range(HC):
            hps = p5_ps.tile([128, 1024], F32, tag="hps")
            for c in range(CC):
                for qb in range(OWN // 512):
                    nc.tensor.matmul(hps[:, qb * 512:(qb + 1) * 512],
                                     w1_sb[c][:, i * 128:(i + 1) * 128],
                                     h2T[c][:, qb * 512:(qb + 1) * 512],
                                     start=(c == 0), stop=(c == CC - 1))
            nc.scalar.activation(out=mT[i], in_=hps, func=AF.Gelu,
                                 bias=b1_sb[:, i:i + 1])
        if debug_dumps:
            nc.gpsimd.dma_start(out=dbg["m0"], in_=mT[0])
        p5_ps.release()
        w1_pool.release()
        h2T_pool.release()

        # ============ P6: MLP down + final residual ============
        p6_ps = tc.alloc_tile_pool(name="p6ps", bufs=2, space="PSUM")
        p6_sb = tc.alloc_tile_pool(name="p6sb", bufs=3)
        for t in range(T8):
            mps = p6_ps.tile([128, 1024], F32, tag="mps")
            for i in range(HC):
                nc.tensor.matmul(mps[:, 0:512],
                                 mT[i][:, t * 128:(t + 1) * 128],
                                 w2_sb[i][:, 0:512],
                                 start=(i == 0), stop=(i == HC - 1))
                nc.tensor.matmul(mps[:, 512:768],
                                 mT[i][:, t * 128:(t + 1) * 128],
                                 w2_sb[i][:, 512:768],
                                 start=(i == 0), stop=(i == HC - 1))
            o_t = p6_sb.tile([128, C], F32, tag="out")
            nc.vector.tensor_add(o_t, y[t], mps[:, 0:C])
            if has_b2:
                nc.vector.tensor_add(o_t, o_t, b2_sb)
            nc.sync.dma_start(out=out_d[t * 128:(t + 1) * 128, :], in_=o_t)
        p6_ps.release()
        p6_sb.release()
        w2_pool.release()
        m_pool.release()
        y_pool.release()
        consts.release()

    nc.compile()
    return nc


def build_in_maps(x, ln1_g, ln1_b, w_qkv, w_proj, b_proj, ln2_g, ln2_b,
                  w1, b1, w2, b2):
    x = np.asarray(x, np.float32)
    ln1_g = np.asarray(ln1_g, np.float32)
    ln1_b = np.asarray(ln1_b, np.float32)
    w_qkv = np.asarray(w_qkv, np.float32)
    w_proj = np.asarray(w_proj, np.float32)
    b_proj = np.asarray(b_proj, np.float32)
    ln2_g = np.asarray(ln2_g, np.float32)
    ln2_b = np.asarray(ln2_b, np.float32)
    w1 = np.asarray(w1, np.float32)
    b1 = np.asarray(b1, np.float32)
    w2 = np.asarray(w2, np.float32)
    b2 = np.asarray(b2, np.float32)

    bf = ml_dtypes.bfloat16
    wqkv_eff = w_qkv * ln1_g[:, None]
    bqkv = ln1_b @ w_qkv
    wq = np.ascontiguousarray(wqkv_eff[:, 0:C]).astype(bf)
    wk = np.ascontiguousarray(wqkv_eff[:, C:2 * C]).astype(bf)
    wv = np.ascontiguousarray(wqkv_eff[:, 2 * C:3 * C]).astype(bf)
    bq = np.ascontiguousarray(bqkv[0:C])
    bk = np.ascontiguousarray(bqkv[C:2 * C])
    bv = np.ascontiguousarray(bqkv[2 * C:3 * C])
    bias2 = bv @ w_proj + b_proj
    w1_eff = (w1 * ln2_g[:, None]).astype(bf)
    b1_eff = b1 + ln2_b @ w1
    wp = w_proj.astype(bf)
    w2b = w2.astype(bf)

    has_bqk = bool(np.any(bq != 0) or np.any(bk != 0))
    has_bias2 = bool(np.any(bias2 != 0))
    has_b2 = bool(np.any(b2 != 0))

    common = {
        "wq": wq, "wk": wk, "wv": wv, "wproj": wp,
        "w1": w1_eff, "w2": w2b,
        "bq": bq, "bk": bk, "b1e": b1_eff,
        "bias2": bias2.astype(np.float32), "b2t": b2,
    }
    in_maps = []
    for c in range(NCORES):
        b, half = divmod(c, 2)
        if half == 0:
            xseq = x[b]
        else:
            xseq = np.concatenate([x[b][OWN:], x[b][:OWN]], axis=0)
        m = dict(common)
        m["xseq"] = np.ascontiguousarray(xseq)
        in_maps.append(m)
    return in_maps, (has_bqk, has_bias2, has_b2)


def kernel(**inputs):
    in_maps, key = build_in_maps(**inputs)
    if key not in _CACHE:
        _CACHE[key] = _build_program(*key)
    nc = _CACHE[key]
    res = run_bass_kernel_spmd(nc, in_maps, core_ids=list(range(NCORES)))
    out = np.empty((B, N, C), np.float32)
    for c in range(NCORES):
        b, half = divmod(c, 2)
        out[b, half * OWN:(half + 1) * OWN, :] = res.results[c]["out"]
    return out


# revision 27
# speedup vs baseline: 202.5691x; 1.0178x over previous
"""Trainium2 Bass kernel for a ViT-style transformer block (nn_Block).

Reference computation (per batch sequence):
    h   = LN(x) * g1 + b1
    qkv = h @ w_qkv ; attention (12 heads, dh=64, softmax over keys)
    x   = x + (attn_out @ w_proj + b_proj)
    h2  = LN(x) * g2 + b2
    out = x + gelu(h2 @ w1 + b1) @ w2 + b2

Sharding: 8 cores; core c handles half of sequence b=c//2 (1024 query
tokens).  The input sequence is host-rolled so each core's own tokens are
always rows [0:1024] (keeps the SPMD program identical across cores).
K/V are computed on-device for the full 2048-token sequence (redundantly
per core pair) so no collectives are needed.

Layout strategy (all matmul operands bf16, fp32 PSUM accumulation,
fp32 residual/LN/softmax paths):
  hT  [C=6x128, tok]   <- LN1 in token-major + PE transpose
  qT/kT per head pair [128 (2 heads x 64), tok]
  V'  [tok, 12, 65]    (65th column = ones -> PV matmul yields row sums)
  S^T [keys=128, 2h x 2qb x 512] per (head-pair, key-tile); exp on ACT
  O^T [65, 1024] accumulated over key tiles; row 64 = softmax denominators
  proj/MLP in token-major with PE transposes after LN2.

LN gains/biases are folded into the weights host-side (exact).
"""

import numpy as np
import ml_dtypes

import concourse.bass as bass
import concourse.tile as tile
from concourse import bacc, mybir
from concourse.bass_utils import run_bass_kernel_spmd
from concourse.masks import make_identity

F32 = mybir.dt.float32
BF16 = mybir.dt.bfloat16
AF = mybir.ActivationFunctionType

B, N, C = 4, 2048, 768
H, DH = 12, 64
HID = 4 * C
EPS = 1e-5
NCORES = 8

SEQ = N            # tokens per core's sequence (full, for K/V)
OWN = N // 2       # own query tokens per core
T16 = SEQ // 128   # token tiles (full seq)
T8 = OWN // 128    # token tiles (own)
CC = C // 128      # C chunks (6)
HP = H // 2        # head pairs (6)
HC = HID // 128    # hidden chunks (24)

_CACHE = {}


def _build_program(has_bqk: bool, has_bias2: bool, has_b2: bool,
                   debug_dumps: bool = False):
    nc = bacc.Bacc("TRN2", target_bir_lowering=False, debug=False,
                   num_devices=NCORES)
    dbg = {}
    if debug_dumps:
        dbg["hT"] = nc.dram_tensor("d_hT", [CC, 128, SEQ], F32,
                                   kind="ExternalOutput").ap()
        dbg["kT0"] = nc.dram_tensor("d_kT0", [128, SEQ], F32,
                                    kind="ExternalOutput").ap()
        dbg["qT0"] = nc.dram_tensor("d_qT0", [128, OWN], F32,
                                    kind="ExternalOutput").ap()
        dbg["v0"] = nc.dram_tensor("d_v0", [128, H, DH + 1], F32,
                                   kind="ExternalOutput").ap()
        dbg["pt00"] = nc.dram_tensor("d_pt00", [128, 1024], F32,
                                     kind="ExternalOutput").ap()
        dbg["oT0"] = nc.dram_tensor("d_oT0", [128, OWN], F32,
                                    kind="ExternalOutput").ap()
        dbg["y0"] = nc.dram_tensor("d_y0", [128, C], F32,
                                   kind="ExternalOutput").ap()
        dbg["h2T0"] = nc.dram_tensor("d_h2T0", [128, OWN], F32,
                                     kind="ExternalOutput").ap()
        dbg["m0"] = nc.dram_tensor("d_m0", [128, OWN], F32,
                                   kind="ExternalOutput").ap()

    # ---------------- DRAM I/O ----------------
    x_d = nc.dram_tensor("xseq", [SEQ, C], F32, kind="ExternalInput").ap()
    wq_d = nc.dram_tensor("wq", [C, C], BF16, kind="ExternalInput").ap()
    wk_d = nc.dram_tensor("wk", [C, C], BF16, kind="ExternalInput").ap()
    wv_d = nc.dram_tensor("wv", [C, C], BF16, kind="ExternalInput").ap()
    wp_d = nc.dram_tensor("wproj", [C, C], BF16, kind="ExternalInput").ap()
    w1_d = nc.dram_tensor("w1", [C, HID], BF16, kind="ExternalInput").ap()
    w2_d = nc.dram_tensor("w2", [HID, C], BF16, kind="ExternalInput").ap()
    bq_d = nc.dram_tensor("bq", [C], F32, kind="ExternalInput").ap()
    bk_d = nc.dram_tensor("bk", [C], F32, kind="ExternalInput").ap()
    b1_d = nc.dram_tensor("b1e", [HID], F32, kind="ExternalInput").ap()
    bias2_d = nc.dram_tensor("bias2", [C], F32, kind="ExternalInput").ap()
    b2_d = nc.dram_tensor("b2t", [C], F32, kind="ExternalInput").ap()
    out_d = nc.dram_tensor("out", [OWN, C], F32, kind="ExternalOutput").ap()

    def bcast_row(dram_ap, parts=128):
        # DRAM [n] -> SBUF [parts, n] partition-broadcast AP
        return bass.AP(tensor=dram_ap.tensor, offset=dram_ap.offset,
                       ap=[[0, parts]] + list(dram_ap.ap))

    with tile.TileContext(nc) as tc:
        # ---------------- persistent constants ----------------
        consts = tc.alloc_tile_pool(name="consts", bufs=1)
        ident = consts.tile([128, 128], BF16, tag="ident")
        make_identity(nc, ident)
        eps_t = consts.tile([128, 1], F32, tag="eps")
        nc.vector.memset(eps_t, EPS)
        bq_sb = consts.tile([128, CC], F32, tag="bq")
        nc.gpsimd.dma_start(out=bq_sb, in_=bq_d.rearrange("(a p) -> p a", p=128))
        bk_sb = consts.tile([128, CC], F32, tag="bk")
        nc.gpsimd.dma_start(out=bk_sb, in_=bk_d.rearrange("(a p) -> p a", p=128))
        b1_sb = consts.tile([128, HC], F32, tag="b1")
        nc.gpsimd.dma_start(out=b1_sb, in_=b1_d.rearrange("(a p) -> p a", p=128))
        if has_bias2:
            bias2_sb = consts.tile([128, C], F32, tag="bias2")
            nc.gpsimd.dma_start(out=bias2_sb, in_=bcast_row(bias2_d))
        if has_b2:
            b2_sb = consts.tile([128, C], F32, tag="b2")
            nc.gpsimd.dma_start(out=b2_sb, in_=bcast_row(b2_d))

        # persistent activation pools (phase-crossing lifetimes)
        # SBUF pool stacks must pop LIFO per side; long-lived pools are
        # ordered so each side's release order nests properly.
        xres_pool = tc.alloc_tile_pool(name="xres", bufs=1)
        xres = [xres_pool.tile([128, C], F32, tag=f"xr{t}", name=f"xr{t}") for t in range(T8)]
        hT_pool = tc.alloc_tile_pool(name="hT", bufs=1)
        hT = [[hT_pool.tile([128, 512], BF16, tag=f"hT{c}_{nb}",
                            name=f"hT{c}_{nb}")
               for nb in range(SEQ // 512)] for c in range(CC)]

        # weight prefetch: issue QKV weight DMAs before P1 so they
        # overlap the x loads / LN phase (pool sits above hT on the left
        # stack; released at end of P2).
        wqkv_pool = tc.alloc_tile_pool(name="wqkv", bufs=1)
        wq_sb = [wqkv_pool.tile([128, C], BF16, tag=f"wq{c}", name=f"wq{c}") for c in range(CC)]
        wk_sb = [wqkv_pool.tile([128, C], BF16, tag=f"wk{c}", name=f"wk{c}") for c in range(CC)]
        wv_sb = [wqkv_pool.tile([128, C], BF16, tag=f"wv{c}", name=f"wv{c}") for c in range(CC)]
        for c in range(CC):
            nc.sync.dma_start(out=wq_sb[c], in_=wq_d[c * 128:(c + 1) * 128, :])
            nc.sync.dma_start(out=wk_sb[c], in_=wk_d[c * 128:(c + 1) * 128, :])
            nc.sync.dma_start(out=wv_sb[c], in_=wv_d[c * 128:(c + 1) * 128, :])


        # ============ P1: LN1 + transpose -> hT ============
        p1_ps = tc.alloc_tile_pool(name="p1ps", bufs=4, space="PSUM", side="right")
        p1_sb = tc.alloc_tile_pool(name="p1sb", bufs=4)
        for t in range(T16):
            if t < T8:
                x_t = xres[t]
            else:
                x_t = p1_sb.tile([128, C], F32, tag="xin")
            nc.sync.dma_start(out=x_t, in_=x_d[t * 128:(t + 1) * 128, :])
            st = p1_sb.tile([128, 3, 6], F32, tag="stats")
            xg = x_t.rearrange("p (n s) -> p n s", s=256)
            for i in range(3):
                nc.vector.bn_stats(out=st[:, i, :], in_=xg[:, i, :])
            mv = p1_sb.tile([128, 2], F32, tag="mv")
            nc.vector.bn_aggr(out=mv, in_=st)
            rstd = p1_sb.tile([128, 1], F32, tag="rstd")
            nc.scalar.activation(out=rstd, in_=mv[:, 1:2], func=AF.Sqrt,
                                 bias=eps_t)
            nc.vector.reciprocal(out=rstd, in_=rstd)
            h_t = p1_sb.tile([128, C], BF16, tag="h")
            nc.vector.tensor_scalar(out=h_t, in0=x_t, scalar1=mv[:, 0:1],
                                    scalar2=rstd,
                                    op0=mybir.AluOpType.subtract,
                                    op1=mybir.AluOpType.mult)
            for c in range(CC):
                tp = p1_ps.tile([128, 128], BF16, tag="tp")
                nc.tensor.transpose(tp, h_t[:, c * 128:(c + 1) * 128], ident)
                dst = hT[c][t // 4][:, (t % 4) * 128:(t % 4 + 1) * 128]
                if c % 3 == 0:
                    nc.scalar.copy(out=dst, in_=tp)
                else:
                    nc.vector.tensor_copy(out=dst, in_=tp)
        p1_sb.release()
        p1_ps.release()
        if debug_dumps:
            for c in range(CC):
                nc.gpsimd.dma_start(out=dbg["hT"][c], in_=hT[c])

        # ============ P2: QKV ============
        kv_pool = tc.alloc_tile_pool(name="kv", bufs=1, side="right")
        kT = [kv_pool.tile([128, SEQ], BF16, tag=f"kT{p}", name=f"kT{p}") for p in range(HP)]
        qT = [kv_pool.tile([128, OWN], BF16, tag=f"qT{p}", name=f"qT{p}") for p in range(HP)]
        vP = [kv_pool.tile([128, H, DH + 1], BF16, tag=f"v{t}", name=f"v{t}")
              for t in range(T16)]

        # --- kT (full sequence) ---
        p2k_ps = tc.alloc_tile_pool(name="p2kps", bufs=2, space="PSUM")
        for p in range(HP):
            for half in range(2):
                kps = p2k_ps.tile([128, 1024], F32, tag="kps")
                for c in range(CC):
                    for nb in range(2):
                        nc.tensor.matmul(
                            kps[:, nb * 512:(nb + 1) * 512],
                            wk_sb[c][:, p * 128:(p + 1) * 128],
                            hT[c][half * 2 + nb],
                            start=(c == 0), stop=(c == CC - 1))
                dst = kT[p][:, half * 1024:(half + 1) * 1024]
                if has_bqk:
                    nc.vector.tensor_scalar(out=dst, in0=kps,
                                            scalar1=bk_sb[:, p:p + 1],
                                            op0=mybir.AluOpType.add)
                else:
                    nc.scalar.copy(out=dst, in_=kps)
        p2k_ps.release()

        # --- qT (own tokens) + V' ---
        p2q_ps = tc.alloc_tile_pool(name="p2qps", bufs=2, space="PSUM")
        for p in range(HP):
            qps = p2q_ps.tile([128, OWN], F32, tag="qps")
            for c in range(CC):
                for nb in range(OWN // 512):
                    nc.tensor.matmul(qps[:, nb * 512:(nb + 1) * 512],
                                     wq_sb[c][:, p * 128:(p + 1) * 128],
                                     hT[c][nb],
                                     start=(c == 0), stop=(c == CC - 1))
            if has_bqk:
                nc.vector.tensor_scalar(out=qT[p], in0=qps,
                                        scalar1=bq_sb[:, p:p + 1],
                                        op0=mybir.AluOpType.add)
            else:
                nc.scalar.copy(out=qT[p], in_=qps)
        p2q_ps.release()

        if debug_dumps:
            nc.gpsimd.dma_start(out=dbg["kT0"], in_=kT[0])
            nc.gpsimd.dma_start(out=dbg["qT0"], in_=qT[0])
        # ============ P3: attention ============
        oT_pool = tc.alloc_tile_pool(name="oT", bufs=1)
        oT = [oT_pool.tile([128, OWN], BF16, tag=f"oT{p}", name=f"oT{p}") for p in range(HP)]

        # prefetch proj weights during attention (DMA idle there)
        wp_pool = tc.alloc_tile_pool(name="wp", bufs=1)
        wp_sb = [wp_pool.tile([128, C], BF16, tag=f"wp{p}", name=f"wp{p}") for p in range(HP)]
        for p in range(HP):
            nc.sync.dma_start(out=wp_sb[p], in_=wp_d[p * 128:(p + 1) * 128, :])

        s_ps = tc.alloc_tile_pool(name="sps", bufs=2, space="PSUM")
        o_ps = tc.alloc_tile_pool(name="ops", bufs=1, space="PSUM")
        v_ps = tc.alloc_tile_pool(name="vps", bufs=1, space="PSUM")
        pt_pool = tc.alloc_tile_pool(name="pt", bufs=4)
        sm_pool = tc.alloc_tile_pool(name="sm", bufs=2)

        # V' production emitted here: its matmuls fill the PE slack while
        # the attention loop below keeps ACT saturated with exp ops.
        for t in range(T16):
            vps = v_ps.tile([128, 1024], F32, tag="vps")
            for c in range(CC):
                hsl = hT[c][t // 4][:, (t % 4) * 128:(t % 4 + 1) * 128]
                nc.tensor.matmul(vps[:, 0:512], hsl, wv_sb[c][:, 0:512],
                                 start=(c == 0), stop=(c == CC - 1))
                nc.tensor.matmul(vps[:, 512:768], hsl, wv_sb[c][:, 512:768],
                                 start=(c == 0), stop=(c == CC - 1))
            nc.vector.tensor_copy(
                out=vP[t][:, :, 0:DH],
                in_=vps[:, 0:C].rearrange("p (g d) -> p g d", d=DH))
            nc.vector.memset(vP[t][:, :, DH:DH + 1], 1.0)

        for p in range(HP):
            for qb in range(OWN // 512):
                ops = [o_ps.tile([65, 512], F32, tag=f"o{h}", name=f"o{h}")
                       for h in range(2)]
                for kt in range(T16):
                    sps = s_ps.tile([128, 1024], F32, tag="s", name="sps")
                    for h in range(2):
                        nc.tensor.matmul(
                            sps[:, h * 512:(h + 1) * 512],
                            kT[p][h * 64:(h + 1) * 64,
                                  kt * 128:(kt + 1) * 128],
                            qT[p][h * 64:(h + 1) * 64,
                                  qb * 512:(qb + 1) * 512],
                            start=True, stop=True)
                    ptt = pt_pool.tile([128, 1024], BF16, tag="pt", name="ptt")
                    nc.scalar.activation(out=ptt, in_=sps, func=AF.Exp,
                                         scale=float(DH) ** -0.5)
                    for h in range(2):
                        g = p * 2 + h
                        nc.tensor.matmul(
                            ops[h][:, 0:512],
                            vP[kt][:, g, :],
                            ptt[:, h * 512:(h + 1) * 512],
                            start=(kt == 0), stop=(kt == T16 - 1))
                # softmax normalization for this (head pair, query block):
                # copy O off PSUM immediately; recip + partition-broadcast +
                # multiply run on DVE/GPSIMD/DMA off the critical path.
                for h in range(2):
                    ofull = sm_pool.tile([65, 512], F32, tag="ofl", name="ofl")
                    nc.vector.tensor_copy(out=ofull, in_=ops[h][0:65, :])
                    rec = sm_pool.tile([65, 512], F32, tag="rec", name="rec")
                    nc.vector.reciprocal_approx_fast(rec, ofull)
                    rrow = sm_pool.tile([1, 512], F32, tag="rrow", name="rrow")
                    nc.sync.dma_start(out=rrow, in_=rec[64:65, :])
                    bcast = sm_pool.tile([64, 512], F32, tag="bcast",
                                         name="bcast")
                    nc.gpsimd.partition_broadcast(bcast, rrow[0:1, :])
                    if h == 0:
                        nc.vector.tensor_mul(
                            oT[p][0:64, qb * 512:(qb + 1) * 512],
                            ofull[0:64, :], bcast)
                    else:
                        tmp = sm_pool.tile([64, 512], BF16, tag="otmp",
                                           name="otmp")
                        nc.vector.tensor_mul(tmp, ofull[0:64, :], bcast)
                        nc.sync.dma_start(
                            out=oT[p][64:128, qb * 512:(qb + 1) * 512],
                            in_=tmp)
        v_ps.release()
        o_ps.release()
        s_ps.release()
        sm_pool.release()
        pt_pool.release()
        kv_pool.release()

        # ============ P4: proj + residual + LN2 + h2T ============
        y_pool = tc.alloc_tile_pool(name="y", bufs=1, side="right")
        y = [y_pool.tile([128, C], F32, tag=f"y{t}", name=f"y{t}") for t in range(T8)]
        h2T_pool = tc.alloc_tile_pool(name="h2T", bufs=1, side="right")
        h2T = [h2T_pool.tile([128, OWN], BF16, tag=f"h2T{c}", name=f"h2T{c}") for c in range(CC)]
        # prefetch MLP-up weights during proj/LN2 (right side: reuses the
        # K/Q/V space released at end of attention)
        w1_pool = tc.alloc_tile_pool(name="w1p", bufs=1, side="right")
        w1_sb = [w1_pool.tile([128, HID], BF16, tag=f"w1{c}", name=f"w1{c}") for c in range(CC)]
        for c in range(CC):
            nc.sync.dma_start(out=w1_sb[c], in_=w1_d[c * 128:(c + 1) * 128, :])

        p4_ps = tc.alloc_tile_pool(name="p4ps", bufs=2, space="PSUM")
        p4t_ps = tc.alloc_tile_pool(name="p4tps", bufs=4, space="PSUM")
        p4_sb = tc.alloc_tile_pool(name="p4sb", bufs=3)
        for t in range(T8):
            aps = p4_ps.tile([128, 1024], F32, tag="aps")
            for p in range(HP):
                nc.tensor.matmul(aps[:, 0:512],
                                 oT[p][:, t * 128:(t + 1) * 128],
                                 wp_sb[p][:, 0:512],
                                 start=(p == 0), stop=(p == HP - 1))
                nc.tensor.matmul(aps[:, 512:768],
                                 oT[p][:, t * 128:(t + 1) * 128],
                                 wp_sb[p][:, 512:768],
                                 start=(p == 0), stop=(p == HP - 1))
            nc.vector.tensor_add(y[t], xres[t], aps[:, 0:C])
            if has_bias2:
                nc.vector.tensor_add(y[t], y[t], bias2_sb)
            st = p4_sb.tile([128, 3, 6], F32, tag="stats")
            yg = y[t].rearrange("p (n s) -> p n s", s=256)
            for i in range(3):
                nc.vector.bn_stats(out=st[:, i, :], in_=yg[:, i, :])
            mv = p4_sb.tile([128, 2], F32, tag="mv")
            nc.vector.bn_aggr(out=mv, in_=st)
            rstd = p4_sb.tile([128, 1], F32, tag="rstd")
            nc.scalar.activation(out=rstd, in_=mv[:, 1:2], func=AF.Sqrt,
                                 bias=eps_t)
            nc.vector.reciprocal(out=rstd, in_=rstd)
            h2 = p4_sb.tile([128, C], BF16, tag="h2")
            nc.vector.tensor_scalar(out=h2, in0=y[t], scalar1=mv[:, 0:1],
                                    scalar2=rstd,
                                    op0=mybir.AluOpType.subtract,
                                    op1=mybir.AluOpType.mult)
            for c in range(CC):
                tp = p4t_ps.tile([128, 128], BF16, tag="tp")
                nc.tensor.transpose(tp, h2[:, c * 128:(c + 1) * 128], ident)
                nc.scalar.copy(out=h2T[c][:, t * 128:(t + 1) * 128], in_=tp)
        if debug_dumps:
            nc.gpsimd.dma_start(out=dbg["y0"], in_=y[0])
            nc.gpsimd.dma_start(out=dbg["h2T0"], in_=h2T[0])
        p4_sb.release()
        p4t_ps.release()
        p4_ps.release()
        wp_pool.release()
        oT_pool.release()
        wqkv_pool.release()
        hT_pool.release()
        xres_pool.release()

        # ============ P5: MLP up + gelu ============
        m_pool = tc.alloc_tile_pool(name="m", bufs=1)
        mT = [m_pool.tile([128, OWN], BF16, tag=f"m{i}", name=f"m{i}") for i in range(HC)]
        w2_pool = tc.alloc_tile_pool(name="w2p", bufs=1)
        w2_sb = [w2_pool.tile([128, C], BF16, tag=f"w2{i}", name=f"w2{i}") for i in range(HC)]
        for i in range(HC):
            nc.sync.dma_start(out=w2_sb[i], in_=w2_d[i * 128:(i + 1) * 128, :])


        p5_ps = tc.alloc_tile_pool(name="p5ps", bufs=3, space="PSUM")
        for i in range(HC):
            hps = p5_ps.tile([128, 1024], F32, tag="hps")
            for c in range(CC):
                for qb in range(OWN // 512):
                    nc.tensor.matmul(hps[:, qb * 512:(qb + 1) * 512],
                                     w1_sb[c][:, i * 128:(i + 1) * 128],
                                     h2T[c][:, qb * 512:(qb + 1) * 512],
                                     start=(c == 0), stop=(c == CC - 1))
            nc.scalar.activation(out=mT[i], in_=hps, func=AF.Gelu,
                                 bias=b1_sb[:, i:i + 1])
        if debug_dumps:
            nc.gpsimd.dma_start(out=dbg["m0"], in_=mT[0])
        p5_ps.release()
        w1_pool.release()
        h2T_pool.release()

        # ============ P6: MLP down + final residual ============
        p6_ps = tc.alloc_tile_pool(name="p6ps", bufs=2, space="PSUM")
        p6_sb = tc.alloc_tile_pool(name="p6sb", bufs=3)
        for t in range(T8):
            mps = p6_ps.tile([128, 1024], F32, tag="mps")
            for i in range(HC):
                nc.tensor.matmul(mps[:, 0:512],
                                 mT[i][:, t * 128:(t + 1) * 128],
                                 w2_sb[i][:, 0:512],
                                 start=(i == 0), stop=(i == HC - 1))
                nc.tensor.matmul(mps[:, 512:768],
                                 mT[i][:, t * 128:(t + 1) * 128],
                                 w2_sb[i][:, 512:768],
                                 start=(i == 0), stop=(i == HC - 1))
            o_t = p6_sb.tile([128, C], F32, tag="out")
            nc.vector.tensor_add(o_t, y[t], mps[:, 0:C])
            if has_b2:
                nc.vector.tensor_add(o_t, o_t, b2_sb)
            nc.sync.dma_start(out=out_d[t * 128:(t + 1) * 128, :], in_=o_t)
        p6_ps.release()
        p6_sb.release()
        w2_pool.release()
        m_pool.release()
        y_pool.release()
        consts.release()

    nc.compile()
    return nc


def build_in_maps(x, ln1_g, ln1_b, w_qkv, w_proj, b_proj, ln2_g, ln2_b,
                  w1, b1, w2, b2):
    x = np.asarray(x, np.float32)
    ln1_g = np.asarray(ln1_g, np.float32)
    ln1_b = np.asarray(ln1_b, np.float32)
    w_qkv = np.asarray(w_qkv, np.float32)
    w_proj = np.asarray(w_proj, np.float32)
    b_proj = np.asarray(b_proj, np.float32)
    ln2_g = np.asarray(ln2_g, np.float32)
    ln2_b = np.asarray(ln2_b, np.float32)
    w1 = np.asarray(w1, np.float32)
    b1 = np.asarray(b1, np.float32)
    w2 = np.asarray(w2, np.float32)
    b2 = np.asarray(b2, np.float32)

    bf = ml_dtypes.bfloat16
    wqkv_eff = w_qkv * ln1_g[:, None]
    bqkv = ln1_b @ w_qkv
    wq = np.ascontiguousarray(wqkv_eff[:, 0:C]).astype(bf)
    wk = np.ascontiguousarray(wqkv_eff[:, C:2 * C]).astype(bf)
    wv = np.ascontiguousarray(wqkv_eff[:, 2 * C:3 * C]).astype(bf)
    bq = np.ascontiguousarray(bqkv[0:C])
    bk = np.ascontiguousarray(bqkv[C:2 * C])
    bv = np.ascontiguousarray(bqkv[2 * C:3 * C])
    bias2 = bv @ w_proj + b_proj
    w1_eff = (w1 * ln2_g[:, None]).astype(bf)
    b1_eff = b1 + ln2_b @ w1
    wp = w_proj.astype(bf)
    w2b = w2.astype(bf)

    has_bqk = bool(np.any(bq != 0) or np.any(bk != 0))
    has_bias2 = bool(np.any(bias2 != 0))
    has_b2 = bool(np.any(b2 != 0))

    common = {
        "wq": wq, "wk": wk, "wv": wv, "wproj": wp,
        "w1": w1_eff, "w2": w2b,
        "bq": bq, "bk": bk, "b1e": b1_eff,
        "bias2": bias2.astype(np.float32), "b2t": b2,
    }
    in_maps = []
    for c in range(NCORES):
        b, half = divmod(c, 2)
        if half == 0:
            xseq = x[b]
        else:
            xseq = np.concatenate([x[b][OWN:], x[b][:OWN]], axis=0)
        m = dict(common)
        m["xseq"] = np.ascontiguousarray(xseq)
        in_maps.append(m)
    return in_maps, (has_bqk, has_bias2, has_b2)


def kernel(**inputs):
    in_maps, key = build_in_maps(**inputs)
    if key not in _CACHE:
        _CACHE[key] = _build_program(*key)
    nc = _CACHE[key]
    res = run_bass_kernel_spmd(nc, in_maps, core_ids=list(range(NCORES)))
    out = np.empty((B, N, C), np.float32)
    for c in range(NCORES):
        b, half = divmod(c, 2)
        out[b, half * OWN:(half + 1) * OWN, :] = res.results[c]["out"]
    return out


# revision 28
# speedup vs baseline: 203.6237x; 1.0052x over previous
"""Trainium2 Bass kernel for a ViT-style transformer block (nn_Block).

Reference computation (per batch sequence):
    h   = LN(x) * g1 + b1
    qkv = h @ w_qkv ; attention (12 heads, dh=64, softmax over keys)
    x   = x + (attn_out @ w_proj + b_proj)
    h2  = LN(x) * g2 + b2
    out = x + gelu(h2 @ w1 + b1) @ w2 + b2

Sharding: 8 cores; core c handles half of sequence b=c//2 (1024 query
tokens).  The input sequence is host-rolled so each core's own tokens are
always rows [0:1024] (keeps the SPMD program identical across cores).
K/V are computed on-device for the full 2048-token sequence (redundantly
per core pair) so no collectives are needed.

Layout strategy (all matmul operands bf16, fp32 PSUM accumulation,
fp32 residual/LN/softmax paths):
  hT  [C=6x128, tok]   <- LN1 in token-major + PE transpose
  qT/kT per head pair [128 (2 heads x 64), tok]
  V'  [tok, 12, 65]    (65th column = ones -> PV matmul yields row sums)
  S^T [keys=128, 2h x 2qb x 512] per (head-pair, key-tile); exp on ACT
  O^T [65, 1024] accumulated over key tiles; row 64 = softmax denominators
  proj/MLP in token-major with PE transposes after LN2.

LN gains/biases are folded into the weights host-side (exact).
"""

import numpy as np
import ml_dtypes

import concourse.bass as bass
import concourse.tile as tile
from concourse import bacc, mybir
from concourse.bass_utils import run_bass_kernel_spmd
from concourse.masks import make_identity

F32 = mybir.dt.float32
BF16 = mybir.dt.bfloat16
AF = mybir.ActivationFunctionType

B, N, C = 4, 2048, 768
H, DH = 12, 64
HID = 4 * C
EPS = 1e-5
NCORES = 8

SEQ = N            # tokens per core's sequence (full, for K/V)
OWN = N // 2       # own query tokens per core
T16 = SEQ // 128   # token tiles (full seq)
T8 = OWN // 128    # token tiles (own)
CC = C // 128      # C chunks (6)
HP = H // 2        # head pairs (6)
HC = HID // 128    # hidden chunks (24)

_CACHE = {}


def _build_program(has_bqk: bool, has_bias2: bool, has_b2: bool,
                   debug_dumps: bool = False):
    nc = bacc.Bacc("TRN2", target_bir_lowering=False, debug=False,
                   num_devices=NCORES)
    dbg = {}
    if debug_dumps:
        dbg["hT"] = nc.dram_tensor("d_hT", [CC, 128, SEQ], F32,
                                   kind="ExternalOutput").ap()
        dbg["kT0"] = nc.dram_tensor("d_kT0", [128, SEQ], F32,
                                    kind="ExternalOutput").ap()
        dbg["qT0"] = nc.dram_tensor("d_qT0", [128, OWN], F32,
                                    kind="ExternalOutput").ap()
        dbg["v0"] = nc.dram_tensor("d_v0", [128, H, DH + 1], F32,
                                   kind="ExternalOutput").ap()
        dbg["pt00"] = nc.dram_tensor("d_pt00", [128, 1024], F32,
                                     kind="ExternalOutput").ap()
        dbg["oT0"] = nc.dram_tensor("d_oT0", [128, OWN], F32,
                                    kind="ExternalOutput").ap()
        dbg["y0"] = nc.dram_tensor("d_y0", [128, C], F32,
                                   kind="ExternalOutput").ap()
        dbg["h2T0"] = nc.dram_tensor("d_h2T0", [128, OWN], F32,
                                     kind="ExternalOutput").ap()
        dbg["m0"] = nc.dram_tensor("d_m0", [128, OWN], F32,
                                   kind="ExternalOutput").ap()

    # ---------------- DRAM I/O ----------------
    x_d = nc.dram_tensor("xseq", [SEQ, C], F32, kind="ExternalInput").ap()
    wq_d = nc.dram_tensor("wq", [C, C], BF16, kind="ExternalInput").ap()
    wk_d = nc.dram_tensor("wk", [C, C], BF16, kind="ExternalInput").ap()
    wv_d = nc.dram_tensor("wv", [C, C], BF16, kind="ExternalInput").ap()
    wp_d = nc.dram_tensor("wproj", [C, C], BF16, kind="ExternalInput").ap()
    w1_d = nc.dram_tensor("w1", [C, HID], BF16, kind="ExternalInput").ap()
    w2_d = nc.dram_tensor("w2", [HID, C], BF16, kind="ExternalInput").ap()
    bq_d = nc.dram_tensor("bq", [C], F32, kind="ExternalInput").ap()
    bk_d = nc.dram_tensor("bk", [C], F32, kind="ExternalInput").ap()
    b1_d = nc.dram_tensor("b1e", [HID], F32, kind="ExternalInput").ap()
    bias2_d = nc.dram_tensor("bias2", [C], F32, kind="ExternalInput").ap()
    b2_d = nc.dram_tensor("b2t", [C], F32, kind="ExternalInput").ap()
    out_d = nc.dram_tensor("out", [OWN, C], F32, kind="ExternalOutput").ap()

    def bcast_row(dram_ap, parts=128):
        # DRAM [n] -> SBUF [parts, n] partition-broadcast AP
        return bass.AP(tensor=dram_ap.tensor, offset=dram_ap.offset,
                       ap=[[0, parts]] + list(dram_ap.ap))

    with tile.TileContext(nc) as tc:
        # ---------------- persistent constants ----------------
        consts = tc.alloc_tile_pool(name="consts", bufs=1)
        ident = consts.tile([128, 128], BF16, tag="ident")
        make_identity(nc, ident)
        eps_t = consts.tile([128, 1], F32, tag="eps")
        nc.vector.memset(eps_t, EPS)
        bq_sb = consts.tile([128, CC], F32, tag="bq")
        nc.gpsimd.dma_start(out=bq_sb, in_=bq_d.rearrange("(a p) -> p a", p=128))
        bk_sb = consts.tile([128, CC], F32, tag="bk")
        nc.gpsimd.dma_start(out=bk_sb, in_=bk_d.rearrange("(a p) -> p a", p=128))
        b1_sb = consts.tile([128, HC], F32, tag="b1")
        nc.gpsimd.dma_start(out=b1_sb, in_=b1_d.rearrange("(a p) -> p a", p=128))
        if has_bias2:
            bias2_sb = consts.tile([128, C], F32, tag="bias2")
            nc.gpsimd.dma_start(out=bias2_sb, in_=bcast_row(bias2_d))
        if has_b2:
            b2_sb = consts.tile([128, C], F32, tag="b2")
            nc.gpsimd.dma_start(out=b2_sb, in_=bcast_row(b2_d))

        # persistent activation pools (phase-crossing lifetimes)
        # SBUF pool stacks must pop LIFO per side; long-lived pools are
        # ordered so each side's release order nests properly.
        xres_pool = tc.alloc_tile_pool(name="xres", bufs=1)
        xres = [xres_pool.tile([128, C], F32, tag=f"xr{t}", name=f"xr{t}") for t in range(T8)]
        hT_pool = tc.alloc_tile_pool(name="hT", bufs=1)
        hT = [[hT_pool.tile([128, 512], BF16, tag=f"hT{c}_{nb}",
                            name=f"hT{c}_{nb}")
               for nb in range(SEQ // 512)] for c in range(CC)]

        # weight prefetch: issue QKV weight DMAs before P1 so they
        # overlap the x loads / LN phase (pool sits above hT on the left
        # stack; released at end of P2).
        wqkv_pool = tc.alloc_tile_pool(name="wqkv", bufs=1)
        wq_sb = [wqkv_pool.tile([128, C], BF16, tag=f"wq{c}", name=f"wq{c}") for c in range(CC)]
        wk_sb = [wqkv_pool.tile([128, C], BF16, tag=f"wk{c}", name=f"wk{c}") for c in range(CC)]
        wv_sb = [wqkv_pool.tile([128, C], BF16, tag=f"wv{c}", name=f"wv{c}") for c in range(CC)]
        for c in range(CC):
            nc.sync.dma_start(out=wq_sb[c], in_=wq_d[c * 128:(c + 1) * 128, :])
            nc.sync.dma_start(out=wk_sb[c], in_=wk_d[c * 128:(c + 1) * 128, :])
            nc.sync.dma_start(out=wv_sb[c], in_=wv_d[c * 128:(c + 1) * 128, :])


        # ============ P1: LN1 + transpose -> hT ============
        p1_ps = tc.alloc_tile_pool(name="p1ps", bufs=4, space="PSUM", side="right")
        p1_sb = tc.alloc_tile_pool(name="p1sb", bufs=6)
        for t in range(T16):
            if t < T8:
                x_t = xres[t]
            else:
                x_t = p1_sb.tile([128, C], F32, tag="xin")
            nc.sync.dma_start(out=x_t, in_=x_d[t * 128:(t + 1) * 128, :])
            st = p1_sb.tile([128, 3, 6], F32, tag="stats")
            xg = x_t.rearrange("p (n s) -> p n s", s=256)
            for i in range(3):
                nc.vector.bn_stats(out=st[:, i, :], in_=xg[:, i, :])
            mv = p1_sb.tile([128, 2], F32, tag="mv")
            nc.vector.bn_aggr(out=mv, in_=st)
            rstd = p1_sb.tile([128, 1], F32, tag="rstd")
            nc.scalar.activation(out=rstd, in_=mv[:, 1:2], func=AF.Sqrt,
                                 bias=eps_t)
            nc.vector.reciprocal(out=rstd, in_=rstd)
            h_t = p1_sb.tile([128, C], BF16, tag="h")
            nc.vector.tensor_scalar(out=h_t, in0=x_t, scalar1=mv[:, 0:1],
                                    scalar2=rstd,
                                    op0=mybir.AluOpType.subtract,
                                    op1=mybir.AluOpType.mult)
            for c in range(CC):
                tp = p1_ps.tile([128, 128], BF16, tag="tp")
                nc.tensor.transpose(tp, h_t[:, c * 128:(c + 1) * 128], ident)
                dst = hT[c][t // 4][:, (t % 4) * 128:(t % 4 + 1) * 128]
                if c % 2 == 0:
                    nc.scalar.copy(out=dst, in_=tp)
                else:
                    nc.vector.tensor_copy(out=dst, in_=tp)
        p1_sb.release()
        p1_ps.release()
        if debug_dumps:
            for c in range(CC):
                nc.gpsimd.dma_start(out=dbg["hT"][c], in_=hT[c])

        # ============ P2: QKV ============
        kv_pool = tc.alloc_tile_pool(name="kv", bufs=1, side="right")
        kT = [kv_pool.tile([128, SEQ], BF16, tag=f"kT{p}", name=f"kT{p}") for p in range(HP)]
        qT = [kv_pool.tile([128, OWN], BF16, tag=f"qT{p}", name=f"qT{p}") for p in range(HP)]
        vP = [kv_pool.tile([128, H, DH + 1], BF16, tag=f"v{t}", name=f"v{t}")
              for t in range(T16)]

        # --- kT (full sequence) ---
        p2k_ps = tc.alloc_tile_pool(name="p2kps", bufs=2, space="PSUM")
        for p in range(HP):
            for half in range(2):
                kps = p2k_ps.tile([128, 1024], F32, tag="kps")
                for c in range(CC):
                    for nb in range(2):
                        nc.tensor.matmul(
                            kps[:, nb * 512:(nb + 1) * 512],
                            wk_sb[c][:, p * 128:(p + 1) * 128],
                            hT[c][half * 2 + nb],
                            start=(c == 0), stop=(c == CC - 1))
                dst = kT[p][:, half * 1024:(half + 1) * 1024]
                if has_bqk:
                    nc.vector.tensor_scalar(out=dst, in0=kps,
                                            scalar1=bk_sb[:, p:p + 1],
                                            op0=mybir.AluOpType.add)
                else:
                    nc.scalar.copy(out=dst, in_=kps)
        p2k_ps.release()

        # --- qT (own tokens) + V' ---
        p2q_ps = tc.alloc_tile_pool(name="p2qps", bufs=2, space="PSUM")
        for p in range(HP):
            qps = p2q_ps.tile([128, OWN], F32, tag="qps")
            for c in range(CC):
                for nb in range(OWN // 512):
                    nc.tensor.matmul(qps[:, nb * 512:(nb + 1) * 512],
                                     wq_sb[c][:, p * 128:(p + 1) * 128],
                                     hT[c][nb],
                                     start=(c == 0), stop=(c == CC - 1))
            if has_bqk:
                nc.vector.tensor_scalar(out=qT[p], in0=qps,
                                        scalar1=bq_sb[:, p:p + 1],
                                        op0=mybir.AluOpType.add)
            else:
                nc.scalar.copy(out=qT[p], in_=qps)
        p2q_ps.release()

        if debug_dumps:
            nc.gpsimd.dma_start(out=dbg["kT0"], in_=kT[0])
            nc.gpsimd.dma_start(out=dbg["qT0"], in_=qT[0])
        # ============ P3: attention ============
        oT_pool = tc.alloc_tile_pool(name="oT", bufs=1)
        oT = [oT_pool.tile([128, OWN], BF16, tag=f"oT{p}", name=f"oT{p}") for p in range(HP)]

        # prefetch proj weights during attention (DMA idle there)
        wp_pool = tc.alloc_tile_pool(name="wp", bufs=1)
        wp_sb = [wp_pool.tile([128, C], BF16, tag=f"wp{p}", name=f"wp{p}") for p in range(HP)]
        for p in range(HP):
            nc.sync.dma_start(out=wp_sb[p], in_=wp_d[p * 128:(p + 1) * 128, :])

        s_ps = tc.alloc_tile_pool(name="sps", bufs=2, space="PSUM")
        o_ps = tc.alloc_tile_pool(name="ops", bufs=1, space="PSUM")
        v_ps = tc.alloc_tile_pool(name="vps", bufs=1, space="PSUM")
        pt_pool = tc.alloc_tile_pool(name="pt", bufs=4)
        sm_pool = tc.alloc_tile_pool(name="sm", bufs=2)

        # V' production emitted here: its matmuls fill the PE slack while
        # the attention loop below keeps ACT saturated with exp ops.
        for t in range(T16):
            vps = v_ps.tile([128, 1024], F32, tag="vps")
            for c in range(CC):
                hsl = hT[c][t // 4][:, (t % 4) * 128:(t % 4 + 1) * 128]
                nc.tensor.matmul(vps[:, 0:512], hsl, wv_sb[c][:, 0:512],
                                 start=(c == 0), stop=(c == CC - 1))
                nc.tensor.matmul(vps[:, 512:768], hsl, wv_sb[c][:, 512:768],
                                 start=(c == 0), stop=(c == CC - 1))
            nc.vector.tensor_copy(
                out=vP[t][:, :, 0:DH],
                in_=vps[:, 0:C].rearrange("p (g d) -> p g d", d=DH))
            nc.vector.memset(vP[t][:, :, DH:DH + 1], 1.0)

        for p in range(HP):
            for qb in range(OWN // 512):
                ops = [o_ps.tile([65, 512], F32, tag=f"o{h}", name=f"o{h}")
                       for h in range(2)]
                for kt in range(T16):
                    sps = s_ps.tile([128, 1024], F32, tag="s", name="sps")
                    for h in range(2):
                        nc.tensor.matmul(
                            sps[:, h * 512:(h + 1) * 512],
                            kT[p][h * 64:(h + 1) * 64,
                                  kt * 128:(kt + 1) * 128],
                            qT[p][h * 64:(h + 1) * 64,
                                  qb * 512:(qb + 1) * 512],
                            start=True, stop=True)
                    ptt = pt_pool.tile([128, 1024], BF16, tag="pt", name="ptt")
                    nc.scalar.activation(out=ptt, in_=sps, func=AF.Exp,
                                         scale=float(DH) ** -0.5)
                    for h in range(2):
                        g = p * 2 + h
                        nc.tensor.matmul(
                            ops[h][:, 0:512],
                            vP[kt][:, g, :],
                            ptt[:, h * 512:(h + 1) * 512],
                            start=(kt == 0), stop=(kt == T16 - 1))
                # softmax normalization for this (head pair, query block):
                # copy O off PSUM immediately; recip + partition-broadcast +
                # multiply run on DVE/GPSIMD/DMA off the critical path.
                for h in range(2):
                    ofull = sm_pool.tile([65, 512], F32, tag="ofl", name="ofl")
                    nc.vector.tensor_copy(out=ofull, in_=ops[h][0:65, :])
                    rec = sm_pool.tile([65, 512], F32, tag="rec", name="rec")
                    nc.vector.reciprocal_approx_fast(rec, ofull)
                    rrow = sm_pool.tile([1, 512], F32, tag="rrow", name="rrow")
                    nc.sync.dma_start(out=rrow, in_=rec[64:65, :])
                    bcast = sm_pool.tile([64, 512], F32, tag="bcast",
                                         name="bcast")
                    nc.gpsimd.partition_broadcast(bcast, rrow[0:1, :])
                    if h == 0:
                        nc.vector.tensor_mul(
                            oT[p][0:64, qb * 512:(qb + 1) * 512],
                            ofull[0:64, :], bcast)
                    else:
                        tmp = sm_pool.tile([64, 512], BF16, tag="otmp",
                                           name="otmp")
                        nc.vector.tensor_mul(tmp, ofull[0:64, :], bcast)
                        nc.sync.dma_start(
                            out=oT[p][64:128, qb * 512:(qb + 1) * 512],
                            in_=tmp)
        v_ps.release()
        o_ps.release()
        s_ps.release()
        sm_pool.release()
        pt_pool.release()
        kv_pool.release()

        # ============ P4: proj + residual + LN2 + h2T ============
        y_pool = tc.alloc_tile_pool(name="y", bufs=1, side="right")
        y = [y_pool.tile([128, C], F32, tag=f"y{t}", name=f"y{t}") for t in range(T8)]
        h2T_pool = tc.alloc_tile_pool(name="h2T", bufs=1, side="right")
        h2T = [h2T_pool.tile([128, OWN], BF16, tag=f"h2T{c}", name=f"h2T{c}") for c in range(CC)]
        # prefetch MLP-up weights during proj/LN2 (right side: reuses the
        # K/Q/V space released at end of attention)
        w1_pool = tc.alloc_tile_pool(name="w1p", bufs=1, side="right")
        w1_sb = [w1_pool.tile([128, HID], BF16, tag=f"w1{c}", name=f"w1{c}") for c in range(CC)]
        for c in range(CC):
            nc.sync.dma_start(out=w1_sb[c], in_=w1_d[c * 128:(c + 1) * 128, :])

        p4_ps = tc.alloc_tile_pool(name="p4ps", bufs=2, space="PSUM")
        p4t_ps = tc.alloc_tile_pool(name="p4tps", bufs=4, space="PSUM")
        p4_sb = tc.alloc_tile_pool(name="p4sb", bufs=3)
        for t in range(T8):
            aps = p4_ps.tile([128, 1024], F32, tag="aps")
            for p in range(HP):
                nc.tensor.matmul(aps[:, 0:512],
                                 oT[p][:, t * 128:(t + 1) * 128],
                                 wp_sb[p][:, 0:512],
                                 start=(p == 0), stop=(p == HP - 1))
                nc.tensor.matmul(aps[:, 512:768],
                                 oT[p][:, t * 128:(t + 1) * 128],
                                 wp_sb[p][:, 512:768],
                                 start=(p == 0), stop=(p == HP - 1))
            nc.vector.tensor_add(y[t], xres[t], aps[:, 0:C])
            if has_bias2:
                nc.vector.tensor_add(y[t], y[t], bias2_sb)
            st = p4_sb.tile([128, 3, 6], F32, tag="stats")
            yg = y[t].rearrange("p (n s) -> p n s", s=256)
            for i in range(3):
                nc.vector.bn_stats(out=st[:, i, :], in_=yg[:, i, :])
            mv = p4_sb.tile([128, 2], F32, tag="mv")
            nc.vector.bn_aggr(out=mv, in_=st)
            rstd = p4_sb.tile([128, 1], F32, tag="rstd")
            nc.scalar.activation(out=rstd, in_=mv[:, 1:2], func=AF.Sqrt,
                                 bias=eps_t)
            nc.vector.reciprocal(out=rstd, in_=rstd)
            h2 = p4_sb.tile([128, C], BF16, tag="h2")
            nc.vector.tensor_scalar(out=h2, in0=y[t], scalar1=mv[:, 0:1],
                                    scalar2=rstd,
                                    op0=mybir.AluOpType.subtract,
                                    op1=mybir.AluOpType.mult)
            for c in range(CC):
                tp = p4t_ps.tile([128, 128], BF16, tag="tp")
                nc.tensor.transpose(tp, h2[:, c * 128:(c + 1) * 128], ident)
                nc.scalar.copy(out=h2T[c][:, t * 128:(t + 1) * 128], in_=tp)
        if debug_dumps:
            nc.gpsimd.dma_start(out=dbg["y0"], in_=y[0])
            nc.gpsimd.dma_start(out=dbg["h2T0"], in_=h2T[0])
        p4_sb.release()
        p4t_ps.release()
        p4_ps.release()
        wp_pool.release()
        oT_pool.release()
        wqkv_pool.release()
        hT_pool.release()
        xres_pool.release()

        # ============ P5: MLP up + gelu ============
        m_pool = tc.alloc_tile_pool(name="m", bufs=1)
        mT = [m_pool.tile([128, OWN], BF16, tag=f"m{i}", name=f"m{i}") for i in range(HC)]
        w2_pool = tc.alloc_tile_pool(name="w2p", bufs=1)
        w2_sb = [w2_pool.tile([128, C], BF16, tag=f"w2{i}", name=f"w2{i}") for i in range(HC)]
        for i in range(HC):
            nc.sync.dma_start(out=w2_sb[i], in_=w2_d[i * 128:(i + 1) * 128, :])


        p5_ps = tc.alloc_tile_pool(name="p5ps", bufs=3, space="PSUM")
        for i in range(HC):
            hps = p5_ps.tile([128, 1024], F32, tag="hps")
            for c in range(CC):
                for qb in range(OWN // 512):
                    nc.tensor.matmul(hps[:, qb * 512:(qb + 1) * 512],
                                     w1_sb[c][:, i * 128:(i + 1) * 128],
                                     h2T[c][:, qb * 512:(qb + 1) * 512],
                                     start=(c == 0), stop=(c == CC - 1))
            nc.scalar.activation(out=mT[i], in_=hps, func=AF.Gelu,
                                 bias=b1_sb[:, i:i + 1])
        if debug_dumps:
            nc.gpsimd.dma_start(out=dbg["m0"], in_=mT[0])
        p5_ps.release()
        w1_pool.release()
        h2T_pool.release()

        # ============ P6: MLP down + final residual ============
        p6_ps = tc.alloc_tile_pool(name="p6ps", bufs=2, space="PSUM")
        p6_sb = tc.alloc_tile_pool(name="p6sb", bufs=3)
        for t in range(T8):
            mps = p6_ps.tile([128, 1024], F32, tag="mps")
            for i in range(HC):
                nc.tensor.matmul(mps[:, 0:512],
                                 mT[i][:, t * 128:(t + 1) * 128],
                                 w2_sb[i][:, 0:512],
                                 start=(i == 0), stop=(i == HC - 1))
                nc.tensor.matmul(mps[:, 512:768],
                                 mT[i][:, t * 128:(t + 1) * 128],
                                 w2_sb[i][:, 512:768],
                                 start=(i == 0), stop=(i == HC - 1))
            o_t = p6_sb.tile([128, C], F32, tag="out")
            nc.vector.tensor_add(o_t, y[t], mps[:, 0:C])
            if has_b2:
                nc.vector.tensor_add(o_t, o_t, b2_sb)
            nc.sync.dma_start(out=out_d[t * 128:(t + 1) * 128, :], in_=o_t)
        p6_ps.release()
        p6_sb.release()
        w2_pool.release()
        m_pool.release()
        y_pool.release()
        consts.release()

    nc.compile()
    return nc


def build_in_maps(x, ln1_g, ln1_b, w_qkv, w_proj, b_proj, ln2_g, ln2_b,
                  w1, b1, w2, b2):
    x = np.asarray(x, np.float32)
    ln1_g = np.asarray(ln1_g, np.float32)
    ln1_b = np.asarray(ln1_b, np.float32)
    w_qkv = np.asarray(w_qkv, np.float32)
    w_proj = np.asarray(w_proj, np.float32)
    b_proj = np.asarray(b_proj, np.float32)
    ln2_g = np.asarray(ln2_g, np.float32)
    ln2_b = np.asarray(ln2_b, np.float32)
    w1 = np.asarray(w1, np.float32)
    b1 = np.asarray(b1, np.float32)
    w2 = np.asarray(w2, np.float32)
    b2 = np.asarray(b2, np.float32)

    bf = ml_dtypes.bfloat16
    wqkv_eff = w_qkv * ln1_g[:, None]
    bqkv = ln1_b @ w_qkv
    wq = np.ascontiguousarray(wqkv_eff[:, 0:C]).astype(bf)
    wk = np.ascontiguousarray(wqkv_eff[:, C:2 * C]).astype(bf)
    wv = np.ascontiguousarray(wqkv_eff[:, 2 * C:3 * C]).astype(bf)
    bq = np.ascontiguousarray(bqkv[0:C])
    bk = np.ascontiguousarray(bqkv[C:2 * C])
    bv = np.ascontiguousarray(bqkv[2 * C:3 * C])
    bias2 = bv @ w_proj + b_proj
    w1_eff = (w1 * ln2_g[:, None]).astype(bf)
    b1_eff = b1 + ln2_b @ w1
    wp = w_proj.astype(bf)
    w2b = w2.astype(bf)

    has_bqk = bool(np.any(bq != 0) or np.any(bk != 0))
    has_bias2 = bool(np.any(bias2 != 0))
    has_b2 = bool(np.any(b2 != 0))

    common = {
        "wq": wq, "wk": wk, "wv": wv, "wproj": wp,
        "w1": w1_eff, "w2": w2b,
        "bq": bq, "bk": bk, "b1e": b1_eff,
        "bias2": bias2.astype(np.float32), "b2t": b2,
    }
    in_maps = []
    for c in range(NCORES):
        b, half = divmod(c, 2)
        if half == 0:
            xseq = x[b]
        else:
            xseq = np.concatenate([x[b][OWN:], x[b][:OWN]], axis=0)
        m = dict(common)
        m["xseq"] = np.ascontiguousarray(xseq)
        in_maps.append(m)
    return in_maps, (has_bqk, has_bias2, has_b2)


def kernel(**inputs):
    in_maps, key = build_in_maps(**inputs)
    if key not in _CACHE:
        _CACHE[key] = _build_program(*key)
    nc = _CACHE[key]
    res = run_bass_kernel_spmd(nc, in_maps, core_ids=list(range(NCORES)))
    out = np.empty((B, N, C), np.float32)
    for c in range(NCORES):
        b, half = divmod(c, 2)
        out[b, half * OWN:(half + 1) * OWN, :] = res.results[c]["out"]
    return out
